# revision 1
# baseline (speedup 1.0000x reference)
"""CapsNet Trainium2 kernel: conv stack + primary caps + dynamic routing.

Distribution: data-parallel convs (batch 256 -> 32/core), then AllToAll to
i-shard (1152 -> 144/core) the routing; one fused AllReduce per routing
iteration carries the unnormalized class sums + softmax denominators.

Layout conventions on device (per core):
  h     [ic_p 128, ic_t 2, b 32, y 20, x 20]   conv1 out / conv2 in
  u     [oc_p 128, oc_t 2, b 32, pos 36]       conv2 out (oc = cap*32+chw)
  a2a blocks: (cap 8, chw_l 4, b 32, pos 36) per destination shard
  routing rows r = (k, i_l) k-major (k=cap, i_l in [0,144))
  class dims f = (c, o) c-major (f = c*16 + o)
"""

import numpy as np
from contextlib import ExitStack

import concourse.bass as bass
import concourse.tile as tile
from concourse import bacc, mybir
from concourse.bass_utils import run_bass_kernel_spmd
from concourse.masks import make_identity

F32 = mybir.dt.float32
F32R = mybir.dt.float32r
AF = mybir.ActivationFunctionType

N_CORES = 8
B = 256
BL = B // N_CORES          # 32 local batch
NCLS = 10
OCH = 16
NI = 1152                  # (k, i_l) rows per core (8*144)
ISH = 144                  # i per core
NITER = 3
NCH = [(0, 512), (512, 1024), (1024, 1152)]


def AP(t_ap, off, dims):
    return bass.AP(tensor=t_ap.tensor, offset=t_ap.offset + off,
                   ap=[list(d) for d in dims])


def build_program():
    nc = bacc.Bacc("TRN2", target_bir_lowering=False, debug=False,
                   num_devices=N_CORES)

    icold = nc.dram_tensor("icold", [81, BL, 400], F32, kind="ExternalInput")
    w1 = nc.dram_tensor("w1", [81, 256], F32, kind="ExternalInput")
    b1 = nc.dram_tensor("b1", [128, 2], F32, kind="ExternalInput")
    w2 = nc.dram_tensor("w2", [81, 256, 256], F32, kind="ExternalInput")
    b2 = nc.dram_tensor("b2", [128, 2], F32, kind="ExternalInput")
    w2s = nc.dram_tensor("w2s", [NI, 160], F32, kind="ExternalInput")
    w3s = nc.dram_tensor("w3s", [160, NI], F32, kind="ExternalInput")
    ssel = nc.dram_tensor("ssel", [160, NCLS], F32, kind="ExternalInput")
    capsum = nc.dram_tensor("capsum", [128, 2, 8], F32, kind="ExternalInput")
    expnd = nc.dram_tensor("expnd", [8, 2, 128], F32, kind="ExternalInput")
    rexpa = nc.dram_tensor("rexpa", [128, 9, 128], F32, kind="ExternalInput")
    rexpb = nc.dram_tensor("rexpb", [16, 9, 128], F32, kind="ExternalInput")
    y = nc.dram_tensor("y", [B, NCLS, OCH], F32, kind="ExternalOutput")

    with tile.TileContext(nc) as tc, ExitStack() as ctx:
        consts = ctx.enter_context(tc.tile_pool(name="consts", bufs=1))
        dram = ctx.enter_context(tc.tile_pool(name="dram", bufs=1, space="DRAM"))

        b1_sb = consts.tile([128, 2], F32)
        nc.sync.dma_start(b1_sb[:], b1[:])
        b2_sb = consts.tile([128, 2], F32)
        nc.sync.dma_start(b2_sb[:], b2[:])
        ident = consts.tile([128, 128], F32)
        make_identity(nc, ident[:])

        a2a_in = dram.tile([N_CORES, BL, 8, 4, 36], F32)   # (m, b, cap, chw_l, pos)
        a2a_out = dram.tile([N_CORES, BL, 8, 4, 36], F32)  # (src, b_l, cap, chw_l, pos)

        with tc.tile_pool(name="hpool", bufs=1) as hpool, \
             tc.tile_pool(name="upool", bufs=1) as upool:
            h = hpool.tile([128, 2, 32, 20, 20], F32R)
            u = upool.tile([128, 2, BL, 36], F32)

            # ============ conv1: 1->256 k9 s1 + ReLU ============
            with tc.tile_pool(name="icolp", bufs=1) as icolp, \
                 tc.tile_pool(name="ps1", bufs=8, space="PSUM") as ps1:
                icol_r = icolp.tile([81, BL, 400], F32R)
                for bc in range(8):
                    ist = icolp.tile([81, 4, 400], F32, tag="ist", bufs=3,
                                     name=f"ist{bc}")
                    nc.sync.dma_start(ist[:], icold[:][:, 4 * bc:4 * bc + 4, :])
                    nc.vector.tensor_copy(icol_r[:, 4 * bc:4 * bc + 4, :], ist[:])
                w1_sb = icolp.tile([81, 256], F32)
                nc.sync.dma_start(w1_sb[:], w1[:])
                w1_r = icolp.tile([81, 256], F32R)
                nc.vector.tensor_copy(w1_r[:], w1_sb[:])
                for t in range(2):
                    for b in range(BL):
                        p = ps1.tile([128, 400], F32, tag="c1")
                        nc.tensor.matmul(
                            p[:],
                            lhsT=w1_r[:, t * 128:(t + 1) * 128],
                            rhs=icol_r[:, b, :],
                            start=True, stop=True)
                        if b % 2 == 0:
                            nc.scalar.activation(
                                h[:, t, b, :, :],
                                p[:].rearrange("p (y x) -> p y x", y=20),
                                AF.Relu, bias=b1_sb[:, t:t + 1], scale=1.0)
                        else:
                            nc.vector.tensor_scalar(
                                out=h[:, t, b, :, :],
                                in0=p[:].rearrange("p (y x) -> p y x", y=20),
                                scalar1=b1_sb[:, t:t + 1], scalar2=0.0,
                                op0=mybir.AluOpType.add,
                                op1=mybir.AluOpType.max)

            # ============ conv2: 256->256 k9 s2 + bias ============
            hv = h[:]
            with tc.tile_pool(name="w2p", bufs=3) as w2p, \
                 tc.tile_pool(name="ps2", bufs=8, space="PSUM") as ps2:
                psum2 = [ps2.tile([128, 8, 36], F32, tag="c2", name=f"c2_{i}")
                         for i in range(8)]
                for kk in range(81):
                    ky, kx = kk // 9, kk % 9
                    w2t = w2p.tile([128, 2, 256], F32)
                    nc.sync.dma_start(
                        w2t[:],
                        AP(w2[:], kk * 65536, [[256, 128], [32768, 2], [1, 256]]))
                    w2r = w2p.tile([128, 2, 256], F32R)
                    nc.vector.tensor_copy(w2r[:], w2t[:])
                    for ic_t in range(2):
                        for oc_t in range(2):
                            lhs = w2r[:, ic_t, oc_t * 128:(oc_t + 1) * 128]
                            for bc in range(4):
                                rhs = hv[:, ic_t, bc * 8:(bc + 1) * 8,
                                         ky:ky + 12:2, kx:kx + 12:2]
                                nc.tensor.matmul(
                                    psum2[oc_t * 4 + bc][:], lhsT=lhs,
                                    rhs=rhs,
                                    start=(kk == 0 and ic_t == 0),
                                    stop=(kk == 80 and ic_t == 1))
                for oc_t in range(2):
                    for bc in range(4):
                        nc.scalar.activation(
                            u[:, oc_t, bc * 8:(bc + 1) * 8, :],
                            psum2[oc_t * 4 + bc][:], AF.Identity,
                            bias=b2_sb[:, oc_t:oc_t + 1], scale=1.0)

            # ============ squash over i per (b, cap) ============
            with tc.tile_pool(name="sqp", bufs=1) as sqp, \
                 tc.tile_pool(name="ps3", bufs=2, space="PSUM") as ps3:
                capsum_sb = sqp.tile([128, 2, 8], F32)
                nc.sync.dma_start(capsum_sb[:], capsum[:])
                expnd_sb = sqp.tile([8, 2, 128], F32)
                nc.sync.dma_start(expnd_sb[:], expnd[:])

                usq = sqp.tile([128, 2, BL * 36], F32)
                uv2 = u[:].rearrange("p t b q -> p t (b q)")
                pnorm = ps3.tile([8, BL * 36], F32, tag="pn")
                for oc_t in range(2):
                    nc.vector.tensor_mul(usq[:, oc_t, :], uv2[:, oc_t, :],
                                         uv2[:, oc_t, :])
                    for (n0, n1) in NCH:
                        nc.tensor.matmul(
                            pnorm[:, n0:n1],
                            lhsT=capsum_sb[:, oc_t, :],
                            rhs=usq[:, oc_t, n0:n1],
                            start=(oc_t == 0), stop=(oc_t == 1))
                normsq = sqp.tile([8, BL], F32)
                nc.vector.reduce_sum(
                    out=normsq[:],
                    in_=pnorm[:].rearrange("c (b q) -> c b q", q=36),
                    axis=mybir.AxisListType.X)
                scl = sqp.tile([8, BL], F32)
                nc.scalar.sqrt(scl[:], normsq[:])
                nc.vector.tensor_scalar_add(scl[:], scl[:], 1e-10)
                onep = sqp.tile([8, BL], F32)
                nc.vector.tensor_scalar_add(onep[:], normsq[:], 1.0)
                den = sqp.tile([8, BL], F32)
                nc.vector.tensor_mul(den[:], scl[:], onep[:])
                rden = sqp.tile([8, BL], F32)
                nc.vector.reciprocal(rden[:], den[:])
                fac = sqp.tile([8, BL], F32)
                nc.vector.tensor_mul(fac[:], normsq[:], rden[:])
                sfac = sqp.tile([128, 2, BL], F32)
                for oc_t in range(2):
                    pfac = ps3.tile([128, BL], F32, tag="pf")
                    nc.tensor.matmul(pfac[:], lhsT=expnd_sb[:, oc_t, :],
                                     rhs=fac[:], start=True, stop=True)
                    nc.scalar.copy(sfac[:, oc_t, :], pfac[:])
                    nc.vector.tensor_mul(
                        u[:, oc_t, :, :], u[:, oc_t, :, :],
                        AP(sfac[:], oc_t * BL, [[2 * BL, 128], [1, BL], [0, 36]]))

            # ============ AllToAll pack: blocks (b, cap, chw_l, pos) ============
            # oc channels are host-permuted: partition q = (m%4)*32 + cap*4 + chw_l
            for m in range(N_CORES):
                nc.sync.dma_start(
                    AP(a2a_in[:], m * BL * 1152,
                       [[36, 32], [1152, BL], [1, 36]]),
                    u[(m % 4) * 32:(m % 4) * 32 + 32, m // 4, :, :])
        nc.gpsimd.collective_compute(
            "AllToAll", mybir.AluOpType.bypass,
            replica_groups=[list(range(N_CORES))],
            ins=[a2a_in.opt()], outs=[a2a_out.opt()])

        # ============ routing setup ============
        rts = ctx.enter_context(tc.tile_pool(name="rts", bufs=1))
        u3ki = rts.tile([128, 2, 8, 4, 36], F32)   # (b_p, bh, cap, chw_l, pos)
        u2ki = rts.tile([128, 9, 256], F32)        # ((k,i) rows, tile, b)
        w2s_sb = rts.tile([128, 9, 160], F32)
        w3sA = rts.tile([128, NI], F32)
        w3sB = rts.tile([32, NI], F32)
        sselA = rts.tile([128, NCLS], F32)
        sselB = rts.tile([32, NCLS], F32)
        rexpa_sb = rts.tile([128, 9, 128], F32)
        rexpb_sb = rts.tile([16, 9, 128], F32)
        e2A = rts.tile([128, NI], F32R)
        e2B = rts.tile([32, NI], F32R)
        u3r = rts.tile([128, 2, 8, 4, 36], F32R)
        o3r = rts.tile([128, 2, 160], F32R)
        sselAr = rts.tile([128, NCLS], F32R)
        sselBr = rts.tile([32, NCLS], F32R)
        b_sb = rts.tile([NCLS, ISH], F32)
        expb = rts.tile([NCLS, ISH], F32)
        zloc = rts.tile([NCLS, 1], F32)
        expT = rts.tile([128, NCLS], F32)
        expT2 = rts.tile([16, NCLS], F32)
        e8 = rts.tile([128, 9, NCLS], F32)
        ew2 = rts.tile([128, 9, 160], F32)
        stg = rts.tile([128, 2, 160], F32)
        sg = rts.tile([128, 2, 160], F32)
        zrow = rts.tile([128, NCLS], F32)
        rzrow = rts.tile([128, NCLS], F32)
        sqs = rts.tile([128, 2, 160], F32)
        nrm = rts.tile([128, 2, OCH], F32)
        o3 = rts.tile([128, 2, 160], F32)
        uvf = rts.tile([NCLS, ISH], F32)

        nc.sync.dma_start(
            w2s_sb[:], AP(w2s[:], 0, [[160, 128], [128 * 160, 9], [1, 160]]))
        nc.sync.dma_start(w3sA[:], w3s[:][0:128, :])
        nc.sync.dma_start(w3sB[:], w3s[:][128:160, :])
        nc.sync.dma_start(sselA[:], ssel[:][0:128, :])
        nc.sync.dma_start(sselB[:], ssel[:][128:160, :])
        nc.sync.dma_start(
            rexpa_sb[:], AP(rexpa[:], 0, [[9 * 128, 128], [128, 9], [1, 128]]))
        nc.sync.dma_start(
            rexpb_sb[:], AP(rexpb[:], 0, [[9 * 128, 16], [128, 9], [1, 128]]))
        nc.vector.memset(b_sb[:], 0.0)

        # u3ki receive: partition (src, b_l) dense; free (cap, chw_l, pos) dense
        for bh in range(2):
            nc.sync.dma_start(
                u3ki[:, bh, :, :, :],
                AP(a2a_out[:], bh * 128 * 1152, [[1152, 128], [1, 1152]]))
        nc.vector.tensor_copy(u3r[:], u3ki[:])
        nc.vector.tensor_copy(sselAr[:], sselA[:])
        nc.vector.tensor_copy(sselBr[:], sselB[:])
        # transposes -> u2ki rows (k,i)
        with tc.tile_pool(name="psT", bufs=4, space="PSUM") as psT:
            u3v = u3ki[:].rearrange("p h c w q -> p h (c w q)")
            for bh in range(2):
                for t in range(9):
                    pt = psT.tile([128, 128], F32, tag="tr")
                    nc.tensor.transpose(pt[:], u3v[:, bh, t * 128:(t + 1) * 128],
                                        ident[:])
                    nc.scalar.copy(u2ki[:, t, bh * 128:(bh + 1) * 128], pt[:])

        # ============ routing iterations ============
        ar_in = dram.tile([2, 129, 160], F32)
        ar_out = dram.tile([2, 129, 160], F32)
        zz = rts.tile([2, 160], F32)
        nc.vector.memset(zz[:], 0.0)
        for bh in range(2):
            nc.sync.dma_start(ar_in[bh, 128, :], zz[bh:bh + 1, :])
        psS = ctx.enter_context(tc.tile_pool(name="psS", bufs=2, space="PSUM"))
        psM = ctx.enter_context(tc.tile_pool(name="psM", bufs=2, space="PSUM"))
        psU = ctx.enter_context(tc.tile_pool(name="psU", bufs=1, space="PSUM"))

        for it in range(NITER):
            if it == 0:
                # b == 0: exp(b) = 1 everywhere, Z_local = 144
                nc.vector.memset(zloc[:], float(ISH))
            else:
                nc.scalar.activation(expb[:], b_sb[:], AF.Exp)
                nc.vector.reduce_sum(out=zloc[:], in_=expb[:],
                                     axis=mybir.AxisListType.X)
                pt1 = psS.tile([128, NCLS], F32, tag="sm", bufs=1)
                nc.tensor.transpose(pt1[:], expb[:, 0:128], ident[0:NCLS, 0:NCLS])
                nc.scalar.copy(expT[:], pt1[:])
                pt2 = psS.tile([16, NCLS], F32, tag="sm", bufs=1)
                nc.tensor.transpose(pt2[:], expb[:, 128:144], ident[0:NCLS, 0:NCLS])
                nc.scalar.copy(expT2[:], pt2[:])
                pe8 = psS.tile([128, 9, NCLS], F32, tag="e8g", bufs=1)
                for t in range(9):
                    nc.tensor.matmul(pe8[:, t, :], lhsT=rexpa_sb[:, t, :],
                                     rhs=expT[:], start=True, stop=False)
                    nc.tensor.matmul(pe8[:, t, :], lhsT=rexpb_sb[:, t, :],
                                     rhs=expT2[:], start=False, stop=True)
                nc.vector.tensor_mul(
                    ew2[:].rearrange("p t (c o) -> p t c o", c=NCLS),
                    w2s_sb[:].rearrange("p t (c o) -> p t c o", c=NCLS),
                    AP(pe8[:], 0, [[90, 128], [10, 9], [1, 10], [0, 16]]))
            rhs_w = w2s_sb if it == 0 else ew2
            for bh in range(2):
                pst = psM.tile([128, 160], F32, tag="st", bufs=1)
                for t in range(9):
                    nc.tensor.matmul(pst[:],
                                     lhsT=u2ki[:, t, bh * 128:(bh + 1) * 128],
                                     rhs=rhs_w[:, t, :], start=(t == 0), stop=(t == 8))
                nc.scalar.copy(stg[:, bh, :], pst[:])
                nc.sync.dma_start(ar_in[bh, 0:128, :], stg[:, bh, :])
            nc.sync.dma_start(ar_in[0, 128, 0:NCLS], zloc[:])
            nc.gpsimd.collective_compute(
                "AllReduce", mybir.AluOpType.add,
                replica_groups=[list(range(N_CORES))],
                ins=[ar_in.opt()], outs=[ar_out.opt()])
            for bh in range(2):
                nc.sync.dma_start(sg[:, bh, :], ar_out[bh, 0:128, :])
            nc.sync.dma_start(
                zrow[:], AP(ar_out[:], 128 * 160, [[0, 128], [1, NCLS]]))
            nc.vector.reciprocal(rzrow[:], zrow[:])
            nc.vector.tensor_mul(
                sg[:].rearrange("p t (c o) -> p t c o", c=NCLS),
                sg[:].rearrange("p t (c o) -> p t c o", c=NCLS),
                AP(rzrow[:], 0, [[NCLS, 128], [0, 2], [1, 10], [0, 16]]))
            nc.vector.tensor_mul(sqs[:], sg[:].rearrange("p t f -> p (t f)"),
                                 sg[:].rearrange("p t f -> p (t f)"))
            nc.vector.reduce_sum(
                out=nrm[:],
                in_=sqs[:].rearrange("p t (c o) -> p t o c", c=NCLS),
                axis=mybir.AxisListType.X)
            scl2 = rts.tile([128, 2, OCH], F32)
            nc.scalar.sqrt(scl2[:], nrm[:])
            nc.vector.tensor_scalar_add(scl2[:], scl2[:], 1e-10)
            onep2 = rts.tile([128, 2, OCH], F32)
            nc.vector.tensor_scalar_add(onep2[:], nrm[:], 1.0)
            den2 = rts.tile([128, 2, OCH], F32)
            nc.vector.tensor_mul(den2[:], scl2[:], onep2[:])
            rden2 = rts.tile([128, 2, OCH], F32)
            nc.vector.reciprocal(rden2[:], den2[:])
            fac2 = rts.tile([128, 2, OCH], F32)
            nc.vector.tensor_mul(fac2[:], nrm[:], rden2[:])
            nc.vector.tensor_mul(
                o3[:].rearrange("p t (c o) -> p t c o", c=NCLS),
                sg[:].rearrange("p t (c o) -> p t c o", c=NCLS),
                AP(fac2[:], 0, [[2 * OCH, 128], [OCH, 2], [0, 10], [1, OCH]]))

            if it == NITER - 1:
                for bh in range(2):
                    nc.sync.dma_start(
                        AP(y[:], bh * 128 * 160, [[160, 128], [1, 160]]),
                        o3[:, bh, :])
            else:
                nc.vector.tensor_copy(o3r[:], o3[:])
                u3f = u3r[:].rearrange("p h c w q -> p h (c w q)")
                for (n0, n1) in NCH:
                    for mt, msz in ((0, 128), (1, 32)):
                        pA = psM.tile([128, 512], F32, tag="pA")
                        for bh in range(2):
                            nc.tensor.matmul(
                                pA[:msz, 0:n1 - n0],
                                lhsT=o3r[:, bh, mt * 128:mt * 128 + msz],
                                rhs=u3f[:, bh, n0:n1],
                                start=(bh == 0), stop=(bh == 1))
                        e2dst, w3sx = ((e2A, w3sA) if mt == 0 else (e2B, w3sB))
                        nc.vector.tensor_mul(e2dst[:, n0:n1], pA[:msz, 0:n1 - n0],
                                             w3sx[:, n0:n1])
                puv = psU.tile([NCLS, NI], F32, tag="uv")
                for kc, (ssb, e2b) in enumerate(((sselAr, e2A), (sselBr, e2B))):
                    for (n0, n1) in NCH:
                        nc.tensor.matmul(
                            puv[:, n0:n1], lhsT=ssb[:],
                            rhs=e2b[:, n0:n1],
                            start=(kc == 0), stop=(kc == 1))
                puvv = puv[:].rearrange("c (k i) -> c k i", k=8)
                nc.scalar.copy(uvf[:], puvv[:, 0, :])
                for k in range(1, 8):
                    nc.vector.tensor_add(uvf[:], uvf[:], puvv[:, k, :])
                nc.vector.tensor_add(b_sb[:], b_sb[:], uvf[:])

    nc.compile()
    return nc


_CACHE = {}


def _get_program():
    if "nc" not in _CACHE:
        _CACHE["nc"] = build_program()
    return _CACHE["nc"]


def _host_inputs(x, conv_w, conv_b, prim_w, prim_b, digit_w):
    x = np.asarray(x, dtype=np.float32)
    conv_w = np.asarray(conv_w, dtype=np.float32)
    conv_b = np.asarray(conv_b, dtype=np.float32)
    prim_w = np.asarray(prim_w, dtype=np.float32)
    prim_b = np.asarray(prim_b, dtype=np.float32)
    digit_w = np.asarray(digit_w, dtype=np.float32)

    # im2col of x: (B, 1, 28, 28) -> (B, 81, 400) windows
    xi = x.reshape(B, 28, 28)
    s0, s1, s2 = xi.strides
    win = np.lib.stride_tricks.as_strided(
        xi, shape=(B, 9, 9, 20, 20), strides=(s0, s1, s2, s1, s2))
    icold_full = np.ascontiguousarray(
        win.reshape(B, 81, 400).transpose(1, 0, 2))      # (81, B, 400)

    w1 = np.ascontiguousarray(conv_w.reshape(256, 81).T)
    b1 = np.ascontiguousarray(conv_b.reshape(2, 128).T)
    # permute conv2 output channels: oc=(cap*32+chw) -> q_global so that the
    # AllToAll pack per shard m reads 32 contiguous partitions
    oc = np.arange(256)
    cap_, chw = oc // 32, oc % 32
    qg = (chw // 16) * 128 + (chw // 4 % 4) * 32 + cap_ * 4 + (chw % 4)
    perm_inv = np.argsort(qg)
    w2 = np.ascontiguousarray(
        prim_w.reshape(256, 256, 81).transpose(2, 1, 0)[:, :, perm_inv])
    b2 = np.ascontiguousarray(prim_b[perm_inv].reshape(2, 128).T)
    ssel = np.zeros((160, NCLS), np.float32)
    for c in range(NCLS):
        for o in range(OCH):
            ssel[c * OCH + o, c] = 1.0 / B
    capsum = np.zeros((128, 2, 8), np.float32)
    expnd = np.zeros((8, 2, 128), np.float32)
    for p in range(128):
        for oc_t in range(2):
            c_of_p = (p % 32) // 4
            capsum[p, oc_t, c_of_p] = 1.0
            expnd[c_of_p, oc_t, p] = 1.0
    rexpa = np.zeros((128, 9, 128), np.float32)
    rexpb = np.zeros((16, 9, 128), np.float32)
    for t in range(9):
        for p in range(128):
            i_l = (t * 128 + p) % ISH
            if i_l < 128:
                rexpa[i_l, t, p] = 1.0
            else:
                rexpb[i_l - 128, t, p] = 1.0

    in_maps = []
    for m in range(N_CORES):
        dw = digit_w[m * ISH:(m + 1) * ISH]              # (144, 10, 16, 8)
        w2s_h = np.ascontiguousarray(dw.transpose(3, 0, 1, 2).reshape(NI, 160))
        w3s_h = np.ascontiguousarray(dw.transpose(1, 2, 3, 0).reshape(160, NI))
        in_maps.append({
            "icold": np.ascontiguousarray(icold_full[:, m * BL:(m + 1) * BL, :]),
            "w1": w1, "b1": b1, "w2": w2, "b2": b2,
            "w2s": w2s_h, "w3s": w3s_h, "ssel": ssel,
            "capsum": capsum, "expnd": expnd, "rexpa": rexpa, "rexpb": rexpb,
        })
    return in_maps


def kernel(x, conv_w, conv_b, prim_w, prim_b, digit_w, trace=False):
    nc = _get_program()
    in_maps = _host_inputs(x, conv_w, conv_b, prim_w, prim_b, digit_w)
    res = run_bass_kernel_spmd(nc, in_maps, list(range(N_CORES)), trace=trace)
    out = res.results[0]["y"].reshape(B, NCLS, OCH, 1).astype(np.float32)
    if trace:
        return out, res
    return out



# revision 10
# speedup vs baseline: 1.1804x; 1.1804x over previous
"""CapsNet Trainium2 kernel: conv stack + primary caps + dynamic routing.

Distribution: pure batch-parallel (32 images/core), replicated routing
weights. b_ij is kept replicated on every core, so s_j/out are computed
locally per core for its own images; the only communication is one
AllGather of the per-core partial agreement updates u_v per routing
iteration (2 total — the final iteration needs none, each core writes its
own y chunk and the host concatenates).

Layout conventions on device (per core):
  h      [ic_p 128, ic_t 2, b 32, y 20, x 20]     conv1 out / conv2 in (f32r)
  u      [oc_p 128, oc_t 2, b 32, pos 36]          conv2 out, natural oc order
                                                   (oc = t*128+p, k=oc//32, chw=oc%32)
  r      = oc*36 + pos = k*1152 + i,  i = chw*36 + pos   routing row index
  f      = o*10 + c   class dims ordered o-major (enables packed-c DVE APs)
  u_bT   [b 32, r 9216] bf16                       via DRAM roundtrip
  b_ij   [pq 128, posj 9, c 10]  pq = chw*4 + pos%4, posj = pos//4
"""

import numpy as np
import ml_dtypes
from contextlib import ExitStack

import concourse.bass as bass
import concourse.tile as tile
from concourse import bacc, mybir
from concourse.bass_utils import run_bass_kernel_spmd
from concourse.masks import make_identity

F32 = mybir.dt.float32
F32R = mybir.dt.float32r
BF16 = mybir.dt.bfloat16
AF = mybir.ActivationFunctionType

N_CORES = 8
B = 256
BL = B // N_CORES          # 32 local batch
NCLS = 10
OCH = 16
NF = 160                   # (o, c) o-major
NR = 9216                  # routing rows r = (oc, pos)
NI = 1152                  # capsule units per capsule map
NITER = 3


def AP(t_ap, off, dims):
    return bass.AP(tensor=t_ap.tensor, offset=t_ap.offset + off,
                   ap=[list(d) for d in dims])


def build_program():
    nc = bacc.Bacc("TRN2", target_bir_lowering=False, debug=False,
                   num_devices=N_CORES)

    icold = nc.dram_tensor("icold", [81, BL, 400], F32, kind="ExternalInput")
    w1 = nc.dram_tensor("w1", [81, 256], F32, kind="ExternalInput")
    b1 = nc.dram_tensor("b1", [128, 2], F32, kind="ExternalInput")
    w2 = nc.dram_tensor("w2", [81, 256, 256], F32, kind="ExternalInput")
    b2 = nc.dram_tensor("b2", [128, 2], F32, kind="ExternalInput")
    wr = nc.dram_tensor("wr", [128, 2 * 36 * NF], BF16, kind="ExternalInput")
    w3a = nc.dram_tensor("w3a", [128, NR], BF16, kind="ExternalInput")
    w3b = nc.dram_tensor("w3b", [32, NR], BF16, kind="ExternalInput")
    ssa = nc.dram_tensor("ssa", [128, NCLS], BF16, kind="ExternalInput")
    ssb = nc.dram_tensor("ssb", [32, NCLS], BF16, kind="ExternalInput")
    capsum = nc.dram_tensor("capsum", [128, 2, 8], F32, kind="ExternalInput")
    expnd = nc.dram_tensor("expnd", [8, 2, 128], F32, kind="ExternalInput")
    selq = nc.dram_tensor("selq", [128, 4, 128], BF16, kind="ExternalInput")
    ones128 = nc.dram_tensor("ones128", [128, 1], F32, kind="ExternalInput")
    ones1 = nc.dram_tensor("ones1", [1, 128], F32, kind="ExternalInput")
    y = nc.dram_tensor("y", [BL, NF], F32, kind="ExternalOutput")

    with tile.TileContext(nc) as tc, ExitStack() as ctx:
        consts = ctx.enter_context(tc.tile_pool(name="consts", bufs=1))
        dram = ctx.enter_context(tc.tile_pool(name="dram", bufs=1, space="DRAM"))

        b1_sb = consts.tile([128, 2], F32)
        nc.sync.dma_start(b1_sb[:], b1[:])
        b2_sb = consts.tile([128, 2], F32)
        nc.sync.dma_start(b2_sb[:], b2[:])
        capsum_sb = consts.tile([128, 2, 8], F32)
        nc.sync.dma_start(capsum_sb[:], capsum[:])
        expnd_sb = consts.tile([8, 2, 128], F32)
        nc.sync.dma_start(expnd_sb[:], expnd[:])
        selq_sb = consts.tile([128, 4, 128], BF16)
        nc.sync.dma_start(selq_sb[:], selq[:])
        ssa_sb = consts.tile([128, NCLS], BF16)
        nc.sync.dma_start(ssa_sb[:], ssa[:])
        ssb_sb = consts.tile([32, NCLS], BF16)
        nc.sync.dma_start(ssb_sb[:], ssb[:])
        ones128_sb = consts.tile([128, 1], F32)
        nc.sync.dma_start(ones128_sb[:], ones128[:])
        ones1_sb = consts.tile([1, 128], F32)
        nc.sync.dma_start(ones1_sb[:], ones1[:])
        ident = consts.tile([128, 128], F32)
        make_identity(nc, ident[:])

        ustage = dram.tile([BL, NR], BF16)
        agin = dram.tile([128, 90], F32)
        agout = dram.tile([N_CORES, 128, 90], F32)

        upool = ctx.enter_context(tc.tile_pool(name="upool", bufs=1))
        u = upool.tile([128, 2, BL, 36], F32)

        with tc.tile_pool(name="hpool", bufs=1) as hpool:
            h = hpool.tile([128, 2, 32, 20, 20], F32R)

            # ============ conv1: 1->256 k9 s1 + ReLU ============
            with tc.tile_pool(name="icolp", bufs=1) as icolp, \
                 tc.tile_pool(name="ps1", bufs=8, space="PSUM") as ps1:
                icol_r = icolp.tile([81, BL, 400], F32R)
                for bc in range(8):
                    ist = icolp.tile([81, 4, 400], F32, tag="ist", bufs=3,
                                     name=f"ist{bc}")
                    nc.sync.dma_start(ist[:], icold[:][:, 4 * bc:4 * bc + 4, :])
                    nc.vector.tensor_copy(icol_r[:, 4 * bc:4 * bc + 4, :], ist[:])
                w1_sb = icolp.tile([81, 256], F32)
                nc.sync.dma_start(w1_sb[:], w1[:])
                w1_r = icolp.tile([81, 256], F32R)
                nc.vector.tensor_copy(w1_r[:], w1_sb[:])
                for t in range(2):
                    for b in range(BL):
                        p = ps1.tile([128, 400], F32, tag="c1")
                        nc.tensor.matmul(
                            p[:],
                            lhsT=w1_r[:, t * 128:(t + 1) * 128],
                            rhs=icol_r[:, b, :],
                            start=True, stop=True)
                        if b % 2 == 0:
                            nc.scalar.activation(
                                h[:, t, b, :, :],
                                p[:].rearrange("p (y x) -> p y x", y=20),
                                AF.Relu, bias=b1_sb[:, t:t + 1], scale=1.0)
                        else:
                            nc.vector.tensor_scalar(
                                out=h[:, t, b, :, :],
                                in0=p[:].rearrange("p (y x) -> p y x", y=20),
                                scalar1=b1_sb[:, t:t + 1], scalar2=0.0,
                                op0=mybir.AluOpType.add,
                                op1=mybir.AluOpType.max)

            # ============ conv2: 256->256 k9 s2 + bias ============
            hv = h[:]
            with tc.tile_pool(name="w2p", bufs=3) as w2p, \
                 tc.tile_pool(name="ps2", bufs=8, space="PSUM") as ps2:
                psum2 = [ps2.tile([128, 8, 36], F32, tag="c2", name=f"c2_{i}")
                         for i in range(8)]
                for kk in range(81):
                    ky, kx = kk // 9, kk % 9
                    w2t = w2p.tile([128, 2, 256], F32)
                    nc.sync.dma_start(
                        w2t[:],
                        AP(w2[:], kk * 65536, [[256, 128], [32768, 2], [1, 256]]))
                    w2r = w2p.tile([128, 2, 256], F32R)
                    nc.vector.tensor_copy(w2r[:], w2t[:])
                    for ic_t in range(2):
                        for oc_t in range(2):
                            lhs = w2r[:, ic_t, oc_t * 128:(oc_t + 1) * 128]
                            for bc in range(4):
                                rhs = hv[:, ic_t, bc * 8:(bc + 1) * 8,
                                         ky:ky + 12:2, kx:kx + 12:2]
                                nc.tensor.matmul(
                                    psum2[oc_t * 4 + bc][:], lhsT=lhs,
                                    rhs=rhs,
                                    start=(kk == 0 and ic_t == 0),
                                    stop=(kk == 80 and ic_t == 1))
                for oc_t in range(2):
                    for bc in range(4):
                        nc.scalar.activation(
                            u[:, oc_t, bc * 8:(bc + 1) * 8, :],
                            psum2[oc_t * 4 + bc][:], AF.Identity,
                            bias=b2_sb[:, oc_t:oc_t + 1], scale=1.0)
        # h freed here — stream routing weights into the reclaimed SBUF
        rw = ctx.enter_context(tc.tile_pool(name="rw", bufs=1))
        wr_sb = rw.tile([128, 2 * 36 * NF], BF16)
        nc.sync.dma_start(wr_sb[:, 0:5760], wr[:][:, 0:5760])
        nc.sync.dma_start(wr_sb[:, 5760:11520], wr[:][:, 5760:11520])
        w3a_sb = rw.tile([128, NR], BF16)
        nc.sync.dma_start(w3a_sb[:], w3a[:])
        w3b_sb = rw.tile([32, NR], BF16)
        nc.sync.dma_start(w3b_sb[:], w3b[:])
        wrv = wr_sb[:].rearrange("p (t s f) -> p t s f", t=2, s=36)

        rts = ctx.enter_context(tc.tile_pool(name="rts", bufs=1))
        u_bf = rts.tile([128, 2, BL, 36], BF16)

        # ============ squash over i per (b, cap) ============
        with tc.tile_pool(name="sqp", bufs=1) as sqp, \
             tc.tile_pool(name="psq", bufs=1, space="PSUM") as psq:
            usq = sqp.tile([128, 2, BL * 36], F32)
            uv2 = u[:].rearrange("p t b q -> p t (b q)")
            pnorm = psq.tile([8, BL * 36], F32)
            for t in range(2):
                nc.vector.tensor_mul(usq[:, t, :], uv2[:, t, :], uv2[:, t, :])
                for (n0, n1) in ((0, 512), (512, 1024), (1024, 1152)):
                    nc.tensor.matmul(pnorm[:, n0:n1], lhsT=capsum_sb[:, t, :],
                                     rhs=usq[:, t, n0:n1],
                                     start=(t == 0), stop=(t == 1))
            normsq = sqp.tile([8, BL], F32)
            nc.vector.reduce_sum(
                out=normsq[:],
                in_=pnorm[:].rearrange("c (b q) -> c b q", q=36),
                axis=mybir.AxisListType.X)
            scl = sqp.tile([8, BL], F32)
            nc.scalar.sqrt(scl[:], normsq[:])
            nc.vector.tensor_scalar_add(scl[:], scl[:], 1e-10)
            onep = sqp.tile([8, BL], F32)
            nc.vector.tensor_scalar_add(onep[:], normsq[:], 1.0)
            den = sqp.tile([8, BL], F32)
            nc.vector.tensor_mul(den[:], scl[:], onep[:])
            rden = sqp.tile([8, BL], F32)
            nc.vector.reciprocal(rden[:], den[:])
            fac = sqp.tile([8, BL], F32)
            nc.vector.tensor_mul(fac[:], normsq[:], rden[:])
            sfac = sqp.tile([128, 2, BL], F32)
            for t in range(2):
                pfac = psq.tile([128, BL], F32, tag="pf", name=f"pf{t}")
                nc.tensor.matmul(pfac[:], lhsT=expnd_sb[:, t, :],
                                 rhs=fac[:], start=True, stop=True)
                nc.scalar.copy(sfac[:, t, :], pfac[:])
                # u_bf = squashed u in bf16 (single fused scale+cast)
                nc.vector.tensor_mul(
                    u_bf[:, t, :, :], u[:, t, :, :],
                    AP(sfac[:], t * BL, [[2 * BL, 128], [1, BL], [0, 36]]))

        # ============ u_bT via DRAM roundtrip ============
        for t in range(2):
            nc.sync.dma_start(
                AP(ustage[:], t * 4608, [[36, 128], [NR, BL], [1, 36]]),
                u_bf[:, t, :, :])
        u_bT = rts.tile([BL, NR], BF16)
        nc.sync.dma_start(u_bT[:], ustage[:])

        # ============ routing state ============
        b2t = rts.tile([128, 90], F32)
        nc.vector.memset(b2t[:], 0.0)
        e2a = rts.tile([128, NR], BF16)
        e2b = rts.tile([32, NR], BF16)
        ew = rts.tile([128, 2, 36, NF], BF16)
        expb = rts.tile([128, 90], F32)
        ez = rts.tile([128, 90], F32)
        expzb = rts.tile([128, 90], BF16)
        e_sb = rts.tile([128, 360], BF16)
        zrow = rts.tile([1, NCLS], F32)
        rz = rts.tile([1, NCLS], F32)
        s_sb = rts.tile([BL, NF], F32)
        sq2 = rts.tile([BL, NF], F32)
        nrm = rts.tile([BL, OCH], F32)
        scl2 = rts.tile([BL, OCH], F32)
        onep2 = rts.tile([BL, OCH], F32)
        den2 = rts.tile([BL, OCH], F32)
        rden2 = rts.tile([BL, OCH], F32)
        fac2 = rts.tile([BL, OCH], F32)
        o3bf = rts.tile([BL, NF], BF16)
        yout = rts.tile([BL, NF], F32)
        uvs = rts.tile([NCLS, NI], F32)
        uvsb = rts.tile([128, 90], F32)
        gth = rts.tile([128, N_CORES, 90], F32)
        uvred = rts.tile([128, 90], F32)
        gbf0 = rts.tile([128, NI], BF16)
        gbf1 = rts.tile([128, NI], BF16)
        gbfb0 = rts.tile([32, NI], BF16)
        gbfb1 = rts.tile([32, NI], BF16)

        facap = [[OCH, BL], [1, OCH], [0, NCLS]]
        s3ap = [[NF, BL], [NCLS, OCH], [1, NCLS]]

        for it in range(NITER):
            if it > 0:
                # c_ij = softmax(b) over all i; fold 1/Z into the exp table
                with tc.tile_pool(name=f"psE{it}", bufs=1, space="PSUM") as psE:
                    nc.scalar.activation(expb[:], b2t[:], AF.Exp)
                    psz = psE.tile([1, 90], F32)
                    nc.tensor.matmul(psz[:], lhsT=ones128_sb[:], rhs=expb[:],
                                     start=True, stop=True)
                    nc.vector.reduce_sum(
                        out=zrow[:],
                        in_=AP(psz[:], 0, [[90, 1], [1, NCLS], [NCLS, 9]]),
                        axis=mybir.AxisListType.X)
                    nc.vector.reciprocal(rz[:], zrow[:])
                    rz128 = psE.tile([128, NCLS], F32)
                    nc.tensor.matmul(rz128[:], lhsT=ones1_sb[:], rhs=rz[:],
                                     start=True, stop=True)
                    nc.vector.tensor_mul(
                        ez[:], expb[:],
                        AP(rz128[:], 0, [[NCLS, 128], [0, 9], [1, NCLS]]))
                    nc.vector.tensor_copy(expzb[:], ez[:])
                    # expand (pq, posj, c) -> (p, pos, c): e_ps[q][p, posj, c]
                    e_ps = psE.tile([128, 4, 90], F32)
                    for q in range(4):
                        nc.tensor.matmul(e_ps[:, q, :], lhsT=selq_sb[:, q, :],
                                         rhs=expzb[:], start=True, stop=True)
                    nc.scalar.copy(
                        e_sb[:],
                        AP(e_ps[:], 0, [[360, 128], [NCLS, 9], [90, 4], [1, NCLS]]))
                # ew[p, t, pos, o, c] = W * c_ij  (all-bf16 SBUF fast path)
                for t in range(2):
                    nc.vector.tensor_mul(
                        ew[:, t, :, :], wrv[:, t, :, :],
                        AP(e_sb[:], 0, [[360, 128], [NCLS, 36], [0, OCH], [1, NCLS]]))
            rhs_w = wrv if it == 0 else ew[:]

            # s_j for local batch: accumulate over all (oc, pos) rows
            with tc.tile_pool(name=f"psS{it}", bufs=1, space="PSUM") as psS:
                psj = psS.tile([BL, NF], F32)
                for t in range(2):
                    for pos in range(36):
                        nc.tensor.matmul(
                            psj[:], lhsT=u_bf[:, t, :, pos],
                            rhs=rhs_w[:, t, pos, :],
                            start=(t == 0 and pos == 0),
                            stop=(t == 1 and pos == 35))
                nc.scalar.activation(s_sb[:], psj[:], AF.Identity, bias=0.0,
                                     scale=(1.0 / NI) if it == 0 else 1.0)
            # squash over classes per (b, o)
            nc.vector.tensor_mul(sq2[:], s_sb[:], s_sb[:])
            nc.vector.reduce_sum(out=nrm[:], in_=AP(sq2[:], 0, s3ap),
                                 axis=mybir.AxisListType.X)
            nc.scalar.sqrt(scl2[:], nrm[:])
            nc.vector.tensor_scalar_add(scl2[:], scl2[:], 1e-10)
            nc.vector.tensor_scalar_add(onep2[:], nrm[:], 1.0)
            nc.vector.tensor_mul(den2[:], scl2[:], onep2[:])
            nc.vector.reciprocal(rden2[:], den2[:])
            nc.vector.tensor_mul(fac2[:], nrm[:], rden2[:])

            if it == NITER - 1:
                # out in (c, o) order -> y chunk; host concatenates cores
                nc.vector.tensor_mul(
                    AP(yout[:], 0, [[NF, BL], [1, OCH], [OCH, NCLS]]),
                    AP(s_sb[:], 0, [[NF, BL], [NCLS, OCH], [1, NCLS]]),
                    AP(fac2[:], 0, [[OCH, BL], [1, OCH], [0, NCLS]]))
                nc.sync.dma_start(y[:], yout[:])
            else:
                nc.vector.tensor_mul(
                    o3bf[:],
                    AP(s_sb[:], 0, [[NF, BL], [1, NF], [0, 1]]),
                    AP(fac2[:], 0, facap))
                # G^T = sum_b out (x) u ; E = G^T . W3  (per 1152-col chunk)
                with tc.tile_pool(name=f"psG{it}", bufs=2, space="PSUM") as psG:
                    for ci in range(8):
                        sl = slice(ci * NI, (ci + 1) * NI)
                        ga = psG.tile([128, NI], F32, tag="g", name=f"ga{it}_{ci}")
                        for (n0, n1) in ((0, 512), (512, 1024), (1024, 1152)):
                            nc.tensor.matmul(
                                ga[:, n0:n1], lhsT=o3bf[:, 0:128],
                                rhs=u_bT[:, ci * NI + n0:ci * NI + n1],
                                start=True, stop=True)
                        if ci % 2 == 0:
                            nc.vector.tensor_mul(e2a[:, sl], ga[:], w3a_sb[:, sl])
                        else:
                            gbf = gbf0 if ci % 4 == 1 else gbf1
                            nc.scalar.copy(gbf[:], ga[:])
                            nc.vector.tensor_mul(e2a[:, sl], gbf[:], w3a_sb[:, sl])
                        gb = psG.tile([128, NI], F32, tag="g", name=f"gb{it}_{ci}")
                        for (n0, n1) in ((0, 512), (512, 1024), (1024, 1152)):
                            nc.tensor.matmul(
                                gb[0:32, n0:n1], lhsT=o3bf[:, 128:160],
                                rhs=u_bT[:, ci * NI + n0:ci * NI + n1],
                                start=True, stop=True)
                        if ci % 2 == 0:
                            nc.vector.tensor_mul(e2b[:, sl], gb[0:32, :],
                                                 w3b_sb[:, sl])
                        else:
                            gbfb = gbfb0 if ci % 4 == 1 else gbfb1
                            nc.scalar.copy(gbfb[:], gb[0:32, :])
                            nc.vector.tensor_mul(e2b[:, sl], gbfb[:],
                                                 w3b_sb[:, sl])
                # u_v partial: select classes, fold k, transpose to b-layout
                with tc.tile_pool(name=f"psU{it}", bufs=1, space="PSUM") as psU:
                    puv = psU.tile([NCLS, NI], F32)
                    for kc, (ss, e2) in enumerate(((ssa_sb, e2a), (ssb_sb, e2b))):
                        for k in range(8):
                            for (n0, n1) in ((0, 512), (512, 1024), (1024, 1152)):
                                nc.tensor.matmul(
                                    puv[:, n0:n1], lhsT=ss[:],
                                    rhs=e2[:, k * NI + n0:k * NI + n1],
                                    start=(kc == 0 and k == 0),
                                    stop=(kc == 1 and k == 7))
                    # reorder (chw, posj, q) -> (posj, chw, q) during psum copy
                    nc.scalar.copy(
                        uvs[:],
                        AP(puv[:], 0, [[NI, NCLS], [4, 9], [36, 32], [1, 4]]))
                    uvT = psU.tile([128, 9, NCLS], F32)
                    for pj in range(9):
                        nc.tensor.transpose(
                            uvT[:, pj, :],
                            uvs[:, pj * 128:(pj + 1) * 128],
                            ident[0:NCLS, 0:NCLS])
                    nc.vector.tensor_copy(uvsb[:], uvT[:].rearrange("p j c -> p (j c)"))
                nc.sync.dma_start(agin[:], uvsb[:])
                nc.gpsimd.collective_compute(
                    "AllGather", mybir.AluOpType.bypass,
                    replica_groups=[list(range(N_CORES))],
                    ins=[agin.opt()], outs=[agout.opt()])
                nc.sync.dma_start(
                    gth[:], AP(agout[:], 0, [[90, 128], [11520, N_CORES], [1, 90]]))
                nc.vector.reduce_sum(
                    out=uvred[:],
                    in_=AP(gth[:], 0, [[N_CORES * 90, 128], [1, 90], [90, N_CORES]]),
                    axis=mybir.AxisListType.X)
                nc.vector.tensor_add(b2t[:], b2t[:], uvred[:])

    nc.compile()
    return nc


_CACHE = {}


def _get_program():
    if "nc" not in _CACHE:
        _CACHE["nc"] = build_program()
    return _CACHE["nc"]


def _host_inputs(x, conv_w, conv_b, prim_w, prim_b, digit_w):
    x = np.asarray(x, dtype=np.float32)
    conv_w = np.asarray(conv_w, dtype=np.float32)
    conv_b = np.asarray(conv_b, dtype=np.float32)
    prim_w = np.asarray(prim_w, dtype=np.float32)
    prim_b = np.asarray(prim_b, dtype=np.float32)
    digit_w = np.asarray(digit_w, dtype=np.float32)
    bf16 = ml_dtypes.bfloat16

    # im2col of x: (B, 1, 28, 28) -> (B, 81, 400) windows
    xi = x.reshape(B, 28, 28)
    s0, s1, s2 = xi.strides
    win = np.lib.stride_tricks.as_strided(
        xi, shape=(B, 9, 9, 20, 20), strides=(s0, s1, s2, s1, s2))
    icold_full = np.ascontiguousarray(
        win.reshape(B, 81, 400).transpose(1, 0, 2))      # (81, B, 400)

    w1 = np.ascontiguousarray(conv_w.reshape(256, 81).T)
    b1 = np.ascontiguousarray(conv_b.reshape(2, 128).T)
    # natural oc order: oc = t*128+p, k = oc//32, chw = oc%32
    w2 = np.ascontiguousarray(prim_w.reshape(256, 256, 81).transpose(2, 1, 0))
    b2 = np.ascontiguousarray(prim_b.reshape(2, 128).T)

    p_ = np.arange(128)
    # wr[p, t, pos, o, c] = digit_w[i, c, o, k]; i=(p%32)*36+pos, k=(t*128+p)//32
    dw_ikoc = digit_w.transpose(0, 3, 2, 1)              # (i, k, o, c)
    i_pp = (p_[:, None] % 32) * 36 + np.arange(36)[None, :]      # (128, 36)
    k_pt = (np.arange(2)[None, :] * 128 + p_[:, None]) // 32     # (128, 2)
    wr = dw_ikoc[i_pp[:, None, :], k_pt[:, :, None], :, :]       # (128,2,36,16,10)
    wr = np.ascontiguousarray(wr.reshape(128, 2 * 36 * NF)).astype(bf16)

    # w3[f=(o,c), r=(oc,pos)] = digit_w[i(r), c(f), o(f), k(r)]
    r_ = np.arange(NR)
    oc_r = r_ // 36
    i_r = (oc_r % 32) * 36 + (r_ % 36)
    k_r = oc_r // 32
    dw_oci = digit_w.transpose(2, 1, 0, 3)               # (o, c, i, k)
    w3 = dw_oci[:, :, i_r, k_r].reshape(NF, NR)          # (160, 9216)
    w3a = np.ascontiguousarray(w3[0:128]).astype(bf16)
    w3b = np.ascontiguousarray(w3[128:160]).astype(bf16)

    # class selector (sums over o within class, folds 1/B)
    f_ = np.arange(NF)
    ssel = np.zeros((NF, NCLS), np.float32)
    ssel[f_, f_ % NCLS] = 1.0 / B
    ssa = ssel[0:128].astype(bf16)
    ssb = ssel[128:160].astype(bf16)

    capsum = np.zeros((128, 2, 8), np.float32)
    expnd = np.zeros((8, 2, 128), np.float32)
    for t in range(2):
        cap = (t * 128 + p_) // 32
        capsum[p_, t, cap] = 1.0
        expnd[cap, t, p_] = 1.0

    # selq[pq, q, p] = 1 iff pq == (p%32)*4 + q
    selq = np.zeros((128, 4, 128), np.float32)
    for q in range(4):
        selq[(p_ % 32) * 4 + q, q, p_] = 1.0
    selq = selq.astype(bf16)

    ones128 = np.ones((128, 1), np.float32)
    ones1 = np.ones((1, 128), np.float32)

    shared = {
        "w1": w1, "b1": b1, "w2": w2, "b2": b2,
        "wr": wr, "w3a": w3a, "w3b": w3b, "ssa": ssa, "ssb": ssb,
        "capsum": capsum, "expnd": expnd, "selq": selq,
        "ones128": ones128, "ones1": ones1,
    }
    in_maps = []
    for m in range(N_CORES):
        d = dict(shared)
        d["icold"] = np.ascontiguousarray(icold_full[:, m * BL:(m + 1) * BL, :])
        in_maps.append(d)
    return in_maps


def kernel(x, conv_w, conv_b, prim_w, prim_b, digit_w, trace=False):
    nc = _get_program()
    in_maps = _host_inputs(x, conv_w, conv_b, prim_w, prim_b, digit_w)
    res = run_bass_kernel_spmd(nc, in_maps, list(range(N_CORES)), trace=trace)
    out = np.concatenate(
        [np.asarray(res.results[m]["y"]).reshape(BL, NCLS, OCH)
         for m in range(N_CORES)], axis=0)[..., None].astype(np.float32)
    if trace:
        return out, res
    return out


# revision 16
# speedup vs baseline: 1.2392x; 1.0497x over previous
"""CapsNet Trainium2 kernel: conv stack + primary caps + dynamic routing.

Distribution: pure batch-parallel (32 images/core), replicated routing
weights. b_ij is kept replicated on every core, so s_j/out are computed
locally per core for its own images; the only communication is one
AllGather of the per-core partial agreement updates u_v per routing
iteration (2 total — the final iteration needs none, each core writes its
own y chunk and the host concatenates).

Layout conventions on device (per core):
  h      [ic_p 128, ic_t 2, b 32, y 20, x 20]     conv1 out / conv2 in (f32r)
  u      [oc_p 128, oc_t 2, b 32, pos 36]          conv2 out, natural oc order
                                                   (oc = t*128+p, k=oc//32, chw=oc%32)
  r      = oc*36 + pos = k*1152 + i,  i = chw*36 + pos   routing row index
  f      = o*10 + c   class dims ordered o-major (enables packed-c DVE APs)
  u_bT   [b 32, r 9216] bf16                       via DRAM roundtrip
  b_ij   [pq 128, posj 9, c 10]  pq = chw*4 + pos%4, posj = pos//4
"""

import numpy as np
import ml_dtypes
from contextlib import ExitStack

import concourse.bass as bass
import concourse.tile as tile
from concourse import bacc, mybir
from concourse.bass_utils import run_bass_kernel_spmd
from concourse.masks import make_identity

F32 = mybir.dt.float32
F32R = mybir.dt.float32r
BF16 = mybir.dt.bfloat16
AF = mybir.ActivationFunctionType

N_CORES = 8
B = 256
BL = B // N_CORES          # 32 local batch
NCLS = 10
OCH = 16
NF = 160                   # (o, c) o-major
NR = 9216                  # routing rows r = (oc, pos)
NI = 1152                  # capsule units per capsule map
NITER = 3


def AP(t_ap, off, dims):
    return bass.AP(tensor=t_ap.tensor, offset=t_ap.offset + off,
                   ap=[list(d) for d in dims])


def build_program():
    nc = bacc.Bacc("TRN2", target_bir_lowering=False, debug=False,
                   num_devices=N_CORES)

    icold = nc.dram_tensor("icold", [81, BL, 400], F32, kind="ExternalInput")
    w1 = nc.dram_tensor("w1", [81, 256], F32, kind="ExternalInput")
    b1 = nc.dram_tensor("b1", [128, 2], F32, kind="ExternalInput")
    w2 = nc.dram_tensor("w2", [81, 256, 256], F32, kind="ExternalInput")
    b2 = nc.dram_tensor("b2", [128, 2], F32, kind="ExternalInput")
    wr = nc.dram_tensor("wr", [128, 2 * 36 * NF], BF16, kind="ExternalInput")
    w3a = nc.dram_tensor("w3a", [128, NR], BF16, kind="ExternalInput")
    w3b = nc.dram_tensor("w3b", [32, NR], BF16, kind="ExternalInput")
    ssa = nc.dram_tensor("ssa", [128, NCLS], BF16, kind="ExternalInput")
    ssb = nc.dram_tensor("ssb", [32, NCLS], BF16, kind="ExternalInput")
    capsum = nc.dram_tensor("capsum", [128, 2, 8], F32, kind="ExternalInput")
    expnd = nc.dram_tensor("expnd", [8, 2, 128], F32, kind="ExternalInput")
    selq = nc.dram_tensor("selq", [128, 4, 128], BF16, kind="ExternalInput")
    ones128 = nc.dram_tensor("ones128", [128, 1], F32, kind="ExternalInput")
    ones1 = nc.dram_tensor("ones1", [1, 128], F32, kind="ExternalInput")
    y = nc.dram_tensor("y", [BL, NF], F32, kind="ExternalOutput")

    with tile.TileContext(nc) as tc, ExitStack() as ctx:
        consts = ctx.enter_context(tc.tile_pool(name="consts", bufs=1))
        dram = ctx.enter_context(tc.tile_pool(name="dram", bufs=1, space="DRAM"))

        b1_sb = consts.tile([128, 2], F32)
        nc.sync.dma_start(b1_sb[:], b1[:])
        b2_sb = consts.tile([128, 2], F32)
        nc.sync.dma_start(b2_sb[:], b2[:])
        capsum_sb = consts.tile([128, 2, 8], F32)
        nc.sync.dma_start(capsum_sb[:], capsum[:])
        expnd_sb = consts.tile([8, 2, 128], F32)
        nc.sync.dma_start(expnd_sb[:], expnd[:])
        selq_sb = consts.tile([128, 4, 128], BF16)
        nc.sync.dma_start(selq_sb[:], selq[:])
        ssa_sb = consts.tile([128, NCLS], BF16)
        nc.sync.dma_start(ssa_sb[:], ssa[:])
        ssb_sb = consts.tile([32, NCLS], BF16)
        nc.sync.dma_start(ssb_sb[:], ssb[:])
        ones128_sb = consts.tile([128, 1], F32)
        nc.sync.dma_start(ones128_sb[:], ones128[:])
        ones1_sb = consts.tile([1, 128], F32)
        nc.sync.dma_start(ones1_sb[:], ones1[:])
        ident = consts.tile([128, 128], F32)
        make_identity(nc, ident[:])

        ustage = dram.tile([BL, NR], BF16)
        agin = dram.tile([128, 90], BF16)
        agout = dram.tile([N_CORES, 128, 90], BF16)

        upool = ctx.enter_context(tc.tile_pool(name="upool", bufs=1))
        u = upool.tile([128, 2, BL, 36], F32)

        with tc.tile_pool(name="hpool", bufs=1) as hpool:
            h = hpool.tile([128, 2, 32, 20, 20], F32R)

            # ============ conv1: 1->256 k9 s1 + ReLU ============
            with tc.tile_pool(name="icolp", bufs=1) as icolp, \
                 tc.tile_pool(name="ps1", bufs=8, space="PSUM") as ps1:
                icol_r = icolp.tile([81, BL, 400], F32R)
                for bc in range(8):
                    ist = icolp.tile([81, 4, 400], F32, tag="ist", bufs=3,
                                     name=f"ist{bc}")
                    nc.sync.dma_start(ist[:], icold[:][:, 4 * bc:4 * bc + 4, :])
                    nc.vector.tensor_copy(icol_r[:, 4 * bc:4 * bc + 4, :], ist[:])
                w1_sb = icolp.tile([81, 256], F32)
                nc.sync.dma_start(w1_sb[:], w1[:])
                w1_r = icolp.tile([81, 256], F32R)
                nc.vector.tensor_copy(w1_r[:], w1_sb[:])
                for t in range(2):
                    for b in range(BL):
                        p = ps1.tile([128, 400], F32, tag="c1")
                        nc.tensor.matmul(
                            p[:],
                            lhsT=w1_r[:, t * 128:(t + 1) * 128],
                            rhs=icol_r[:, b, :],
                            start=True, stop=True)
                        if b % 2 == 0:
                            nc.scalar.activation(
                                h[:, t, b, :, :],
                                p[:].rearrange("p (y x) -> p y x", y=20),
                                AF.Relu, bias=b1_sb[:, t:t + 1], scale=1.0)
                        else:
                            nc.vector.tensor_scalar(
                                out=h[:, t, b, :, :],
                                in0=p[:].rearrange("p (y x) -> p y x", y=20),
                                scalar1=b1_sb[:, t:t + 1], scalar2=0.0,
                                op0=mybir.AluOpType.add,
                                op1=mybir.AluOpType.max)

            # ============ conv2: 256->256 k9 s2 + bias ============
            hv = h[:]
            with tc.tile_pool(name="w2p", bufs=3) as w2p, \
                 tc.tile_pool(name="ps2", bufs=8, space="PSUM") as ps2:
                psum2 = [ps2.tile([128, 8, 36], F32, tag="c2", name=f"c2_{i}")
                         for i in range(8)]
                for kk in range(81):
                    ky, kx = kk // 9, kk % 9
                    w2t = w2p.tile([128, 2, 256], F32)
                    nc.sync.dma_start(
                        w2t[:],
                        AP(w2[:], kk * 65536, [[256, 128], [32768, 2], [1, 256]]))
                    w2r = w2p.tile([128, 2, 256], F32R)
                    nc.vector.tensor_copy(w2r[:], w2t[:])
                    for ic_t in range(2):
                        for oc_t in range(2):
                            lhs = w2r[:, ic_t, oc_t * 128:(oc_t + 1) * 128]
                            for bc in range(4):
                                rhs = hv[:, ic_t, bc * 8:(bc + 1) * 8,
                                         ky:ky + 12:2, kx:kx + 12:2]
                                nc.tensor.matmul(
                                    psum2[oc_t * 4 + bc][:], lhsT=lhs,
                                    rhs=rhs,
                                    start=(kk == 0 and ic_t == 0),
                                    stop=(kk == 80 and ic_t == 1))
                for oc_t in range(2):
                    for bc in range(4):
                        nc.scalar.activation(
                            u[:, oc_t, bc * 8:(bc + 1) * 8, :],
                            psum2[oc_t * 4 + bc][:], AF.Identity,
                            bias=b2_sb[:, oc_t:oc_t + 1], scale=1.0)
        # h freed here — stream routing weights into the reclaimed SBUF
        rw = ctx.enter_context(tc.tile_pool(name="rw", bufs=1))
        wr_sb = rw.tile([128, 2 * 36 * NF], BF16)
        nc.sync.dma_start(wr_sb[:, 0:5760], wr[:][:, 0:5760])
        nc.sync.dma_start(wr_sb[:, 5760:11520], wr[:][:, 5760:11520])
        w3a_sb = rw.tile([128, NR], BF16)
        nc.sync.dma_start(w3a_sb[:], w3a[:])
        w3b_sb = rw.tile([32, NR], BF16)
        nc.sync.dma_start(w3b_sb[:], w3b[:])
        wrv = wr_sb[:].rearrange("p (t s f) -> p t s f", t=2, s=36)

        rts = ctx.enter_context(tc.tile_pool(name="rts", bufs=1))
        u_bf = rts.tile([128, 2, BL, 36], BF16)

        # ============ squash over i per (b, cap) ============
        with tc.tile_pool(name="sqp", bufs=1) as sqp, \
             tc.tile_pool(name="psq", bufs=1, space="PSUM") as psq:
            usq = sqp.tile([128, 2, BL * 36], F32)
            uv2 = u[:].rearrange("p t b q -> p t (b q)")
            pnorm = psq.tile([8, BL * 36], F32)
            for t in range(2):
                nc.vector.tensor_mul(usq[:, t, :], uv2[:, t, :], uv2[:, t, :])
                for (n0, n1) in ((0, 512), (512, 1024), (1024, 1152)):
                    nc.tensor.matmul(pnorm[:, n0:n1], lhsT=capsum_sb[:, t, :],
                                     rhs=usq[:, t, n0:n1],
                                     start=(t == 0), stop=(t == 1))
            normsq = sqp.tile([8, BL], F32)
            nc.vector.reduce_sum(
                out=normsq[:],
                in_=pnorm[:].rearrange("c (b q) -> c b q", q=36),
                axis=mybir.AxisListType.X)
            # fac = sqrt(n)/(1+n) = exp(0.5*ln(n) - ln(1+n)); avoids the
            # sqrt act-table (stays in the ln/exp table set all routing long)
            lnn = sqp.tile([8, BL], F32)
            nc.scalar.activation(lnn[:], normsq[:], AF.Ln)
            ln1 = sqp.tile([8, BL], F32)
            nc.scalar.activation(ln1[:], normsq[:], AF.Ln, bias=1.0)
            wv = sqp.tile([8, BL], F32)
            nc.vector.tensor_scalar_mul(wv[:], lnn[:], 0.5)
            nc.vector.tensor_sub(wv[:], wv[:], ln1[:])
            fac = sqp.tile([8, BL], F32)
            nc.scalar.activation(fac[:], wv[:], AF.Exp)
            sfac = sqp.tile([128, 2, BL], F32)
            for t in range(2):
                pfac = psq.tile([128, BL], F32, tag="pf", name=f"pf{t}")
                nc.tensor.matmul(pfac[:], lhsT=expnd_sb[:, t, :],
                                 rhs=fac[:], start=True, stop=True)
                nc.scalar.copy(sfac[:, t, :], pfac[:])
                # u_bf = squashed u in bf16 (single fused scale+cast)
                nc.vector.tensor_mul(
                    u_bf[:, t, :, :], u[:, t, :, :],
                    AP(sfac[:], t * BL, [[2 * BL, 128], [1, BL], [0, 36]]))

        # ============ u_bT via DRAM roundtrip ============
        for t in range(2):
            nc.sync.dma_start(
                AP(ustage[:], t * 4608, [[36, 128], [NR, BL], [1, 36]]),
                u_bf[:, t, :, :])
        u_bT = rts.tile([BL, NR], BF16)
        nc.sync.dma_start(u_bT[:], ustage[:])

        # ============ routing state ============
        b2t = rts.tile([128, 90], F32)
        nc.vector.memset(b2t[:], 0.0)
        e2a = rts.tile([128, NR], BF16)
        e2b = rts.tile([32, NR], BF16)
        ew = rts.tile([128, 2, 36, NF], BF16)
        expb = rts.tile([128, 90], F32)
        ez = rts.tile([128, 90], F32)
        expzb = rts.tile([128, 90], BF16)
        e_sb = rts.tile([128, 360], BF16)
        zrow = rts.tile([1, NCLS], F32)
        rz = rts.tile([1, NCLS], F32)
        s_sb = rts.tile([BL, NF], F32)
        sq2 = rts.tile([BL, NF], F32)
        nrm = rts.tile([BL, OCH], F32)
        scl2 = rts.tile([BL, OCH], F32)
        onep2 = rts.tile([BL, OCH], F32)
        den2 = rts.tile([BL, OCH], F32)
        rden2 = rts.tile([BL, OCH], F32)
        fac2 = rts.tile([BL, OCH], F32)
        o3bf = rts.tile([BL, NF], BF16)
        yout = rts.tile([BL, NF], F32)
        uvs = rts.tile([NCLS, NI], F32)
        uvsb = rts.tile([128, 90], BF16)
        gth = rts.tile([128, N_CORES, 90], BF16)
        uvred = rts.tile([128, 90], F32)
        gbf0 = rts.tile([128, NI], BF16)
        gbf1 = rts.tile([128, NI], BF16)
        gbfb0 = rts.tile([32, NI], BF16)
        gbfb1 = rts.tile([32, NI], BF16)

        facap = [[OCH, BL], [1, OCH], [0, NCLS]]
        s3ap = [[NF, BL], [NCLS, OCH], [1, NCLS]]

        for it in range(NITER):
            if it > 0:
                # c_ij = softmax(b) over all i; fold 1/Z into the exp table
                with tc.tile_pool(name=f"psE{it}", bufs=1, space="PSUM") as psE:
                    nc.scalar.activation(expb[:], b2t[:], AF.Exp)
                    psz = psE.tile([1, 90], F32)
                    nc.tensor.matmul(psz[:], lhsT=ones128_sb[:], rhs=expb[:],
                                     start=True, stop=True)
                    nc.vector.reduce_sum(
                        out=zrow[:],
                        in_=AP(psz[:], 0, [[90, 1], [1, NCLS], [NCLS, 9]]),
                        axis=mybir.AxisListType.X)
                    nc.vector.reciprocal(rz[:], zrow[:])
                    rz128 = psE.tile([128, NCLS], F32)
                    nc.tensor.matmul(rz128[:], lhsT=ones1_sb[:], rhs=rz[:],
                                     start=True, stop=True)
                    nc.vector.tensor_mul(
                        expzb[:], expb[:],
                        AP(rz128[:], 0, [[NCLS, 128], [0, 9], [1, NCLS]]))
                    # expand (pq, posj, c) -> (p, pos, c): e_ps[q][p, posj, c]
                    e_ps = psE.tile([128, 4, 90], F32)
                    for q in range(4):
                        nc.tensor.matmul(e_ps[:, q, :], lhsT=selq_sb[:, q, :],
                                         rhs=expzb[:], start=True, stop=True)
                    nc.scalar.copy(
                        e_sb[:],
                        AP(e_ps[:], 0, [[360, 128], [NCLS, 9], [90, 4], [1, NCLS]]))
                # ew[p, t, pos, o, c] = W * c_ij  (all-bf16 SBUF fast path);
                # chunked so the s_j matmuls below start on chunk 0 early
                for t in range(2):
                    for hh in range(2):
                        ps0 = hh * 18
                        nc.vector.tensor_mul(
                            ew[:, t, ps0:ps0 + 18, :],
                            wrv[:, t, ps0:ps0 + 18, :],
                            AP(e_sb[:], ps0 * NCLS,
                               [[360, 128], [NCLS, 18], [0, OCH], [1, NCLS]]))
            rhs_w = wrv if it == 0 else ew[:]

            # s_j for local batch: accumulate over all (oc, pos) rows
            with tc.tile_pool(name=f"psS{it}", bufs=1, space="PSUM") as psS:
                psj = psS.tile([BL, NF], F32)
                for t in range(2):
                    for pos in range(36):
                        nc.tensor.matmul(
                            psj[:], lhsT=u_bf[:, t, :, pos],
                            rhs=rhs_w[:, t, pos, :],
                            start=(t == 0 and pos == 0),
                            stop=(t == 1 and pos == 35))
                nc.vector.tensor_scalar_mul(
                    s_sb[:], psj[:], (1.0 / NI) if it == 0 else 1.0)
            # squash over classes per (b, o): fac = sqrt(n)/(1+n)
            nc.vector.tensor_mul(sq2[:], s_sb[:], s_sb[:])
            nc.vector.reduce_sum(out=nrm[:], in_=AP(sq2[:], 0, s3ap),
                                 axis=mybir.AxisListType.X)
            nc.scalar.activation(scl2[:], nrm[:], AF.Ln)
            nc.scalar.activation(onep2[:], nrm[:], AF.Ln, bias=1.0)
            nc.vector.tensor_scalar_mul(den2[:], scl2[:], 0.5)
            nc.vector.tensor_sub(den2[:], den2[:], onep2[:])
            nc.scalar.activation(fac2[:], den2[:], AF.Exp)

            if it == NITER - 1:
                # out in (c, o) order -> y chunk; host concatenates cores
                nc.vector.tensor_mul(
                    AP(yout[:], 0, [[NF, BL], [1, OCH], [OCH, NCLS]]),
                    AP(s_sb[:], 0, [[NF, BL], [NCLS, OCH], [1, NCLS]]),
                    AP(fac2[:], 0, [[OCH, BL], [1, OCH], [0, NCLS]]))
                nc.sync.dma_start(y[:], yout[:])
            else:
                nc.vector.tensor_mul(
                    o3bf[:],
                    AP(s_sb[:], 0, [[NF, BL], [1, NF], [0, 1]]),
                    AP(fac2[:], 0, facap))
                # G^T = sum_b out (x) u ; E = G^T . W3  (per 1152-col chunk)
                with tc.tile_pool(name=f"psG{it}", bufs=2, space="PSUM") as psG:
                    for ci in range(8):
                        sl = slice(ci * NI, (ci + 1) * NI)
                        ga = psG.tile([128, NI], F32, tag="g", name=f"ga{it}_{ci}")
                        for (n0, n1) in ((0, 512), (512, 1024), (1024, 1152)):
                            nc.tensor.matmul(
                                ga[:, n0:n1], lhsT=o3bf[:, 0:128],
                                rhs=u_bT[:, ci * NI + n0:ci * NI + n1],
                                start=True, stop=True)
                        if ci % 2 == 0:
                            nc.vector.tensor_mul(e2a[:, sl], ga[:], w3a_sb[:, sl])
                        else:
                            gbf = gbf0 if ci % 4 == 1 else gbf1
                            nc.scalar.copy(gbf[:], ga[:])
                            nc.vector.tensor_mul(e2a[:, sl], gbf[:], w3a_sb[:, sl])
                        gb = psG.tile([128, NI], F32, tag="g", name=f"gb{it}_{ci}")
                        for (n0, n1) in ((0, 512), (512, 1024), (1024, 1152)):
                            nc.tensor.matmul(
                                gb[0:32, n0:n1], lhsT=o3bf[:, 128:160],
                                rhs=u_bT[:, ci * NI + n0:ci * NI + n1],
                                start=True, stop=True)
                        if ci % 2 == 0:
                            nc.vector.tensor_mul(e2b[:, sl], gb[0:32, :],
                                                 w3b_sb[:, sl])
                        else:
                            gbfb = gbfb0 if ci % 4 == 1 else gbfb1
                            nc.scalar.copy(gbfb[:], gb[0:32, :])
                            nc.vector.tensor_mul(e2b[:, sl], gbfb[:],
                                                 w3b_sb[:, sl])
                # u_v partial: select classes, fold k, transpose to b-layout
                with tc.tile_pool(name=f"psU{it}", bufs=1, space="PSUM") as psU:
                    puv = psU.tile([NCLS, NI], F32)
                    for kc, (ss, e2) in enumerate(((ssa_sb, e2a), (ssb_sb, e2b))):
                        for k in range(8):
                            for (n0, n1) in ((0, 512), (512, 1024), (1024, 1152)):
                                nc.tensor.matmul(
                                    puv[:, n0:n1], lhsT=ss[:],
                                    rhs=e2[:, k * NI + n0:k * NI + n1],
                                    start=(kc == 0 and k == 0),
                                    stop=(kc == 1 and k == 7))
                    # reorder (chw, posj, q) -> (posj, chw, q) during psum copy
                    nc.scalar.copy(
                        uvs[:],
                        AP(puv[:], 0, [[NI, NCLS], [4, 9], [36, 32], [1, 4]]))
                    uvT = psU.tile([128, 9, NCLS], F32)
                    for pj in range(9):
                        nc.tensor.transpose(
                            uvT[:, pj, :],
                            uvs[:, pj * 128:(pj + 1) * 128],
                            ident[0:NCLS, 0:NCLS])
                    nc.vector.tensor_copy(uvsb[:], uvT[:].rearrange("p j c -> p (j c)"))
                nc.sync.dma_start(agin[:], uvsb[:])
                nc.gpsimd.collective_compute(
                    "AllGather", mybir.AluOpType.bypass,
                    replica_groups=[list(range(N_CORES))],
                    ins=[agin.opt()], outs=[agout.opt()])
                nc.sync.dma_start(
                    gth[:], AP(agout[:], 0, [[90, 128], [11520, N_CORES], [1, 90]]))
                nc.vector.reduce_sum(
                    out=uvred[:],
                    in_=AP(gth[:], 0, [[N_CORES * 90, 128], [1, 90], [90, N_CORES]]),
                    axis=mybir.AxisListType.X)
                nc.vector.tensor_add(b2t[:], b2t[:], uvred[:])

    nc.compile()
    return nc


_CACHE = {}


def _get_program():
    if "nc" not in _CACHE:
        _CACHE["nc"] = build_program()
    return _CACHE["nc"]


def _host_inputs(x, conv_w, conv_b, prim_w, prim_b, digit_w):
    x = np.asarray(x, dtype=np.float32)
    conv_w = np.asarray(conv_w, dtype=np.float32)
    conv_b = np.asarray(conv_b, dtype=np.float32)
    prim_w = np.asarray(prim_w, dtype=np.float32)
    prim_b = np.asarray(prim_b, dtype=np.float32)
    digit_w = np.asarray(digit_w, dtype=np.float32)
    bf16 = ml_dtypes.bfloat16

    # im2col of x: (B, 1, 28, 28) -> (B, 81, 400) windows
    xi = x.reshape(B, 28, 28)
    s0, s1, s2 = xi.strides
    win = np.lib.stride_tricks.as_strided(
        xi, shape=(B, 9, 9, 20, 20), strides=(s0, s1, s2, s1, s2))
    icold_full = np.ascontiguousarray(
        win.reshape(B, 81, 400).transpose(1, 0, 2))      # (81, B, 400)

    w1 = np.ascontiguousarray(conv_w.reshape(256, 81).T)
    b1 = np.ascontiguousarray(conv_b.reshape(2, 128).T)
    # natural oc order: oc = t*128+p, k = oc//32, chw = oc%32
    w2 = np.ascontiguousarray(prim_w.reshape(256, 256, 81).transpose(2, 1, 0))
    b2 = np.ascontiguousarray(prim_b.reshape(2, 128).T)

    p_ = np.arange(128)
    # wr[p, t, pos, o, c] = digit_w[i, c, o, k]; i=(p%32)*36+pos, k=(t*128+p)//32
    dw_ikoc = digit_w.transpose(0, 3, 2, 1)              # (i, k, o, c)
    i_pp = (p_[:, None] % 32) * 36 + np.arange(36)[None, :]      # (128, 36)
    k_pt = (np.arange(2)[None, :] * 128 + p_[:, None]) // 32     # (128, 2)
    wr = dw_ikoc[i_pp[:, None, :], k_pt[:, :, None], :, :]       # (128,2,36,16,10)
    wr = np.ascontiguousarray(wr.reshape(128, 2 * 36 * NF)).astype(bf16)

    # w3[f=(o,c), r=(oc,pos)] = digit_w[i(r), c(f), o(f), k(r)]
    r_ = np.arange(NR)
    oc_r = r_ // 36
    i_r = (oc_r % 32) * 36 + (r_ % 36)
    k_r = oc_r // 32
    dw_oci = digit_w.transpose(2, 1, 0, 3)               # (o, c, i, k)
    w3 = dw_oci[:, :, i_r, k_r].reshape(NF, NR)          # (160, 9216)
    w3a = np.ascontiguousarray(w3[0:128]).astype(bf16)
    w3b = np.ascontiguousarray(w3[128:160]).astype(bf16)

    # class selector (sums over o within class, folds 1/B)
    f_ = np.arange(NF)
    ssel = np.zeros((NF, NCLS), np.float32)
    ssel[f_, f_ % NCLS] = 1.0 / B
    ssa = ssel[0:128].astype(bf16)
    ssb = ssel[128:160].astype(bf16)

    capsum = np.zeros((128, 2, 8), np.float32)
    expnd = np.zeros((8, 2, 128), np.float32)
    for t in range(2):
        cap = (t * 128 + p_) // 32
        capsum[p_, t, cap] = 1.0
        expnd[cap, t, p_] = 1.0

    # selq[pq, q, p] = 1 iff pq == (p%32)*4 + q
    selq = np.zeros((128, 4, 128), np.float32)
    for q in range(4):
        selq[(p_ % 32) * 4 + q, q, p_] = 1.0
    selq = selq.astype(bf16)

    ones128 = np.ones((128, 1), np.float32)
    ones1 = np.ones((1, 128), np.float32)

    shared = {
        "w1": w1, "b1": b1, "w2": w2, "b2": b2,
        "wr": wr, "w3a": w3a, "w3b": w3b, "ssa": ssa, "ssb": ssb,
        "capsum": capsum, "expnd": expnd, "selq": selq,
        "ones128": ones128, "ones1": ones1,
    }
    in_maps = []
    for m in range(N_CORES):
        d = dict(shared)
        d["icold"] = np.ascontiguousarray(icold_full[:, m * BL:(m + 1) * BL, :])
        in_maps.append(d)
    return in_maps


def kernel(x, conv_w, conv_b, prim_w, prim_b, digit_w, trace=False):
    nc = _get_program()
    in_maps = _host_inputs(x, conv_w, conv_b, prim_w, prim_b, digit_w)
    res = run_bass_kernel_spmd(nc, in_maps, list(range(N_CORES)), trace=trace)
    out = np.concatenate(
        [np.asarray(res.results[m]["y"]).reshape(BL, NCLS, OCH)
         for m in range(N_CORES)], axis=0)[..., None].astype(np.float32)
    if trace:
        return out, res
    return out


# revision 30
# speedup vs baseline: 1.2712x; 1.0258x over previous
"""CapsNet Trainium2 kernel: conv stack + primary caps + dynamic routing.

Distribution: pure batch-parallel (32 images/core), replicated routing
weights. b_ij is kept replicated on every core, so s_j/out are computed
locally per core for its own images; the only communication is one
AllGather of the per-core partial agreement updates u_v per routing
iteration (2 total — the final iteration needs none, each core writes its
own y chunk and the host concatenates).

Layout conventions on device (per core):
  h      [ic_p 128, ic_t 2, b 32, y 20, x 20]     conv1 out / conv2 in (f32r)
  u      [oc_p 128, oc_t 2, b 32, pos 36]          conv2 out, natural oc order
                                                   (oc = t*128+p, k=oc//32, chw=oc%32)
  r      = oc*36 + pos = k*1152 + i,  i = chw*36 + pos   routing row index
  f      = o*10 + c   class dims ordered o-major (enables packed-c DVE APs)
  u_bT   [b 32, r 9216] bf16                       via DRAM roundtrip
  b_ij   [pq 128, posj 9, c 10]  pq = chw*4 + pos%4, posj = pos//4
"""

import numpy as np
import ml_dtypes
from contextlib import ExitStack

import concourse.bass as bass
import concourse.tile as tile
from concourse import bacc, mybir
from concourse.bass_utils import run_bass_kernel_spmd
from concourse.masks import make_identity

F32 = mybir.dt.float32
F32R = mybir.dt.float32r
BF16 = mybir.dt.bfloat16
AF = mybir.ActivationFunctionType

N_CORES = 8
B = 256
BL = B // N_CORES          # 32 local batch
NCLS = 10
OCH = 16
NF = 160                   # (o, c) o-major
NR = 9216                  # routing rows r = (oc, pos)
NI = 1152                  # capsule units per capsule map
NITER = 3


def AP(t_ap, off, dims):
    return bass.AP(tensor=t_ap.tensor, offset=t_ap.offset + off,
                   ap=[list(d) for d in dims])


def build_program():
    nc = bacc.Bacc("TRN2", target_bir_lowering=False, debug=False,
                   num_devices=N_CORES)

    icold = nc.dram_tensor("icold", [81, BL, 400], BF16, kind="ExternalInput")
    w1 = nc.dram_tensor("w1", [81, 256], BF16, kind="ExternalInput")
    b1 = nc.dram_tensor("b1", [128, 2], F32, kind="ExternalInput")
    w2 = nc.dram_tensor("w2", [81, 256, 256], F32, kind="ExternalInput")
    b2 = nc.dram_tensor("b2", [128, 2], F32, kind="ExternalInput")
    wr = nc.dram_tensor("wr", [128, 2 * 36 * NF], BF16, kind="ExternalInput")
    w3a = nc.dram_tensor("w3a", [128, NR], BF16, kind="ExternalInput")
    w3b = nc.dram_tensor("w3b", [32, NR], BF16, kind="ExternalInput")
    ssa = nc.dram_tensor("ssa", [128, NCLS], BF16, kind="ExternalInput")
    ssb = nc.dram_tensor("ssb", [32, NCLS], BF16, kind="ExternalInput")
    capsum = nc.dram_tensor("capsum", [128, 2, 8], F32, kind="ExternalInput")
    expnd = nc.dram_tensor("expnd", [8, 2, 128], F32, kind="ExternalInput")
    selq = nc.dram_tensor("selq", [128, 4, 128], BF16, kind="ExternalInput")
    ones128 = nc.dram_tensor("ones128", [128, 1], F32, kind="ExternalInput")
    ones1 = nc.dram_tensor("ones1", [1, 128], F32, kind="ExternalInput")
    y = nc.dram_tensor("y", [BL, NF], F32, kind="ExternalOutput")

    with tile.TileContext(nc) as tc, ExitStack() as ctx:
        consts = ctx.enter_context(tc.tile_pool(name="consts", bufs=1))
        dram = ctx.enter_context(tc.tile_pool(name="dram", bufs=1, space="DRAM"))

        b1_sb = consts.tile([128, 2], F32)
        nc.sync.dma_start(b1_sb[:], b1[:])
        b2_sb = consts.tile([128, 2], F32)
        nc.sync.dma_start(b2_sb[:], b2[:])
        capsum_sb = consts.tile([128, 2, 8], F32)
        nc.sync.dma_start(capsum_sb[:], capsum[:])
        expnd_sb = consts.tile([8, 2, 128], F32)
        nc.sync.dma_start(expnd_sb[:], expnd[:])
        selq_sb = consts.tile([128, 4, 128], BF16)
        nc.sync.dma_start(selq_sb[:], selq[:])
        ssa_sb = consts.tile([128, NCLS], BF16)
        nc.sync.dma_start(ssa_sb[:], ssa[:])
        ssb_sb = consts.tile([32, NCLS], BF16)
        nc.sync.dma_start(ssb_sb[:], ssb[:])
        ones128_sb = consts.tile([128, 1], F32)
        nc.sync.dma_start(ones128_sb[:], ones128[:])
        ones1_sb = consts.tile([1, 128], F32)
        nc.sync.dma_start(ones1_sb[:], ones1[:])
        ident = consts.tile([128, 128], F32)
        make_identity(nc, ident[:])

        ustage = dram.tile([BL, NR], BF16)
        agin = dram.tile([128, 90], BF16)
        agout = dram.tile([N_CORES, 128, 90], BF16)

        upool = ctx.enter_context(tc.tile_pool(name="upool", bufs=1))
        u = upool.tile([128, 2, BL, 36], F32)

        # routing weights: allocated before hpool, streamed during conv
        rw = ctx.enter_context(tc.tile_pool(name="rw", bufs=1))
        wr_sb = rw.tile([128, 2 * 36 * NF], BF16)
        w3a_sb = rw.tile([128, NR], BF16)
        w3b_sb = rw.tile([32, NR], BF16)
        wrv = wr_sb[:].rearrange("p (t s f) -> p t s f", t=2, s=36)

        with tc.tile_pool(name="hpool", bufs=1) as hpool:
            h = hpool.tile([128, 2, 32, 20, 20], F32R)

            # ============ conv1: 1->256 k9 s1 + ReLU (bf16 inputs) ============
            with tc.tile_pool(name="icolp", bufs=1) as icolp, \
                 tc.tile_pool(name="ps1", bufs=1, space="PSUM") as ps1:
                icol_bf = icolp.tile([81, BL, 400], BF16)
                for bc in range(2):
                    nc.sync.dma_start(icol_bf[:, bc * 16:(bc + 1) * 16, :],
                                      icold[:][:, bc * 16:(bc + 1) * 16, :])
                w1_sb = icolp.tile([81, 256], BF16)
                nc.sync.dma_start(w1_sb[:], w1[:])
                # 4-image quads per psum tile -> 4x fewer relu ops
                for t in range(2):
                    for bq in range(8):
                        p = ps1.tile([128, 4, 512], F32, tag="c1", bufs=2,
                                     name=f"c1_{t}_{bq}")
                        for j in range(4):
                            nc.tensor.matmul(
                                p[:, j, 0:400],
                                lhsT=w1_sb[:, t * 128:(t + 1) * 128],
                                rhs=icol_bf[:, bq * 4 + j, :],
                                start=True, stop=True)
                        pin = AP(p[:], 0,
                                 [[2048, 128], [512, 4], [20, 20], [1, 20]])
                        hout = h[:, t, bq * 4:bq * 4 + 4, :, :]
                        if bq % 2 == 0:
                            nc.scalar.activation(
                                hout, pin, AF.Relu,
                                bias=b1_sb[:, t:t + 1], scale=1.0)
                        else:
                            nc.vector.tensor_scalar(
                                out=hout, in0=pin,
                                scalar1=b1_sb[:, t:t + 1], scalar2=0.0,
                                op0=mybir.AluOpType.add,
                                op1=mybir.AluOpType.max)

            # routing weight DMAs (overlap conv2)
            nc.sync.dma_start(wr_sb[:, 0:5760], wr[:][:, 0:5760])
            nc.sync.dma_start(wr_sb[:, 5760:11520], wr[:][:, 5760:11520])
            nc.sync.dma_start(w3a_sb[:], w3a[:])
            nc.sync.dma_start(w3b_sb[:], w3b[:])

            # ============ conv2: 256->256 k9 s2 + bias ============
            hv = h[:]
            with tc.tile_pool(name="w2p", bufs=3) as w2p, \
                 tc.tile_pool(name="ps2", bufs=8, space="PSUM") as ps2:
                psum2 = [ps2.tile([128, 8, 36], F32, tag="c2", name=f"c2_{i}")
                         for i in range(8)]
                for kk in range(81):
                    ky, kx = kk // 9, kk % 9
                    w2t = w2p.tile([128, 2, 256], F32)
                    nc.sync.dma_start(
                        w2t[:],
                        AP(w2[:], kk * 65536, [[256, 128], [32768, 2], [1, 256]]))
                    w2r = w2p.tile([128, 2, 256], F32R)
                    nc.vector.tensor_copy(w2r[:], w2t[:])
                    for ic_t in range(2):
                        for oc_t in range(2):
                            lhs = w2r[:, ic_t, oc_t * 128:(oc_t + 1) * 128]
                            for bc in range(4):
                                rhs = hv[:, ic_t, bc * 8:(bc + 1) * 8,
                                         ky:ky + 12:2, kx:kx + 12:2]
                                nc.tensor.matmul(
                                    psum2[oc_t * 4 + bc][:], lhsT=lhs,
                                    rhs=rhs,
                                    start=(kk == 0 and ic_t == 0),
                                    stop=(kk == 80 and ic_t == 1))
                for oc_t in range(2):
                    for bc in range(4):
                        nc.scalar.activation(
                            u[:, oc_t, bc * 8:(bc + 1) * 8, :],
                            psum2[oc_t * 4 + bc][:], AF.Identity,
                            bias=b2_sb[:, oc_t:oc_t + 1], scale=1.0)
        rts = ctx.enter_context(tc.tile_pool(name="rts", bufs=1))
        u_bf = rts.tile([128, 2, BL, 36], BF16)

        # ============ squash over i per (b, cap) ============
        with tc.tile_pool(name="sqp", bufs=1) as sqp, \
             tc.tile_pool(name="psq", bufs=1, space="PSUM") as psq:
            usq = sqp.tile([128, 2, BL * 36], F32)
            uv2 = u[:].rearrange("p t b q -> p t (b q)")
            pnorm = psq.tile([8, BL * 36], F32)
            for t in range(2):
                nc.vector.tensor_mul(usq[:, t, :], uv2[:, t, :], uv2[:, t, :])
                for (n0, n1) in ((0, 512), (512, 1024), (1024, 1152)):
                    nc.tensor.matmul(pnorm[:, n0:n1], lhsT=capsum_sb[:, t, :],
                                     rhs=usq[:, t, n0:n1],
                                     start=(t == 0), stop=(t == 1))
            normsq = sqp.tile([8, BL], F32)
            nc.vector.reduce_sum(
                out=normsq[:],
                in_=pnorm[:].rearrange("c (b q) -> c b q", q=36),
                axis=mybir.AxisListType.X)
            # fac = sqrt(n)/(1+n) = exp(0.5*ln(n) - ln(1+n)); avoids the
            # sqrt act-table (stays in the ln/exp table set all routing long)
            lnn = sqp.tile([8, BL], F32)
            nc.scalar.activation(lnn[:], normsq[:], AF.Ln)
            ln1 = sqp.tile([8, BL], F32)
            nc.scalar.activation(ln1[:], normsq[:], AF.Ln, bias=1.0)
            wv = sqp.tile([8, BL], F32)
            nc.vector.tensor_scalar_mul(wv[:], lnn[:], 0.5)
            nc.vector.tensor_sub(wv[:], wv[:], ln1[:])
            fac = sqp.tile([8, BL], F32)
            nc.scalar.activation(fac[:], wv[:], AF.Exp)
            sfac = sqp.tile([128, 2, BL], F32)
            for t in range(2):
                pfac = psq.tile([128, BL], F32, tag="pf", name=f"pf{t}")
                nc.tensor.matmul(pfac[:], lhsT=expnd_sb[:, t, :],
                                 rhs=fac[:], start=True, stop=True)
                nc.scalar.copy(sfac[:, t, :], pfac[:])
                # u_bf = squashed u in bf16 (single fused scale+cast)
                nc.vector.tensor_mul(
                    u_bf[:, t, :, :], u[:, t, :, :],
                    AP(sfac[:], t * BL, [[2 * BL, 128], [1, BL], [0, 36]]))

        # ============ u_bT via DRAM roundtrip ============
        for t in range(2):
            nc.sync.dma_start(
                AP(ustage[:], t * 4608, [[36, 128], [NR, BL], [1, 36]]),
                u_bf[:, t, :, :])
        u_bT = rts.tile([BL, NR], BF16)
        nc.sync.dma_start(u_bT[:], ustage[:])

        # ============ routing state ============
        b2t = rts.tile([128, 90], F32)
        nc.vector.memset(b2t[:], 0.0)
        e2a = rts.tile([128, NR], BF16)
        e2b = rts.tile([32, NR], BF16)
        ew = rts.tile([128, 2, 36, NF], BF16)
        expb = rts.tile([128, 90], F32)
        ez = rts.tile([128, 90], F32)
        expzb = rts.tile([128, 90], BF16)
        e_sb = rts.tile([128, 360], BF16)
        zrow = rts.tile([1, NCLS], F32)
        rz = rts.tile([1, NCLS], F32)
        rz32s = rts.tile([BL, NCLS], F32)
        s_sb = rts.tile([BL, NF], F32)
        sq2 = rts.tile([BL, NF], F32)
        nrm = rts.tile([BL, OCH], F32)
        scl2 = rts.tile([BL, OCH], F32)
        onep2 = rts.tile([BL, OCH], F32)
        den2 = rts.tile([BL, OCH], F32)
        rden2 = rts.tile([BL, OCH], F32)
        fac2 = rts.tile([BL, OCH], F32)
        o3bf = rts.tile([BL, NF], BF16)
        yout = rts.tile([BL, NF], F32)
        uvs = rts.tile([NCLS, NI], F32)
        uvsb = rts.tile([128, 90], BF16)
        gth = rts.tile([128, N_CORES, 90], BF16)
        uvred = rts.tile([128, 90], F32)
        gbf0 = rts.tile([128, NI], BF16)
        gbf1 = rts.tile([128, NI], BF16)
        gbfb0 = rts.tile([32, NI], BF16)
        gbfb1 = rts.tile([32, NI], BF16)

        facap = [[OCH, BL], [1, OCH], [0, NCLS]]
        s3ap = [[NF, BL], [NCLS, OCH], [1, NCLS]]

        for it in range(NITER):
            if it > 0:
                # c_ij = softmax(b) over all i; fold 1/Z into the exp table
                with tc.tile_pool(name=f"psE{it}", bufs=1, space="PSUM") as psE:
                    nc.scalar.activation(expb[:], b2t[:], AF.Exp)
                    psz = psE.tile([1, 90], F32)
                    nc.tensor.matmul(psz[:], lhsT=ones128_sb[:], rhs=expb[:],
                                     start=True, stop=True)
                    nc.vector.reduce_sum(
                        out=zrow[:],
                        in_=AP(psz[:], 0, [[90, 1], [1, NCLS], [NCLS, 9]]),
                        axis=mybir.AxisListType.X)
                    nc.vector.reciprocal(rz[:], zrow[:])
                    # keep exp(b) unnormalized; 1/Z folds into the s_j readout
                    nc.vector.tensor_copy(expzb[:], expb[:])
                    # expand (pq, posj, c) -> (p, pos, c): e_ps[q][p, posj, c]
                    e_ps = psE.tile([128, 4, 90], F32)
                    for q in range(4):
                        nc.tensor.matmul(e_ps[:, q, :], lhsT=selq_sb[:, q, :],
                                         rhs=expzb[:], start=True, stop=True)
                    nc.scalar.copy(
                        e_sb[:],
                        AP(e_ps[:], 0, [[360, 128], [NCLS, 9], [90, 4], [1, NCLS]]))
                # ew[p, t, pos, o, c] = W * c_ij  (all-bf16 SBUF fast path);
                # chunked so the s_j matmuls below start on chunk 0 early
                for t in range(2):
                    for hh in range(2):
                        ps0 = hh * 18
                        nc.vector.tensor_mul(
                            ew[:, t, ps0:ps0 + 18, :],
                            wrv[:, t, ps0:ps0 + 18, :],
                            AP(e_sb[:], ps0 * NCLS,
                               [[360, 128], [NCLS, 18], [0, OCH], [1, NCLS]]))
            rhs_w = wrv if it == 0 else ew[:]

            # s_j for local batch: accumulate over all (oc, pos) rows
            with tc.tile_pool(name=f"psS{it}", bufs=1, space="PSUM") as psS:
                psj = psS.tile([BL, NF], F32)
                if it > 0:
                    rz32 = psS.tile([BL, NCLS], F32)
                    nc.tensor.matmul(rz32[:], lhsT=ones1_sb[0:1, 0:BL],
                                     rhs=rz[:], start=True, stop=True)
                    nc.scalar.copy(rz32s[:], rz32[:])
                for t in range(2):
                    for pos in range(36):
                        nc.tensor.matmul(
                            psj[:], lhsT=u_bf[:, t, :, pos],
                            rhs=rhs_w[:, t, pos, :],
                            start=(t == 0 and pos == 0),
                            stop=(t == 1 and pos == 35))
                if it == 0:
                    nc.vector.tensor_scalar_mul(s_sb[:], psj[:], 1.0 / NI)
                else:
                    nc.vector.tensor_mul(
                        s_sb[:], psj[:],
                        AP(rz32s[:], 0, [[NCLS, BL], [0, OCH], [1, NCLS]]))
            # squash over classes per (b, o): fac = sqrt(n)/(1+n)
            nc.vector.tensor_mul(sq2[:], s_sb[:], s_sb[:])
            nc.vector.reduce_sum(out=nrm[:], in_=AP(sq2[:], 0, s3ap),
                                 axis=mybir.AxisListType.X)
            nc.scalar.activation(scl2[:], nrm[:], AF.Ln)
            nc.scalar.activation(onep2[:], nrm[:], AF.Ln, bias=1.0)
            nc.vector.tensor_scalar_mul(den2[:], scl2[:], 0.5)
            nc.vector.tensor_sub(den2[:], den2[:], onep2[:])
            nc.scalar.activation(fac2[:], den2[:], AF.Exp)

            if it == NITER - 1:
                # out in (c, o) order -> y chunk; host concatenates cores
                nc.vector.tensor_mul(
                    AP(yout[:], 0, [[NF, BL], [1, OCH], [OCH, NCLS]]),
                    AP(s_sb[:], 0, [[NF, BL], [NCLS, OCH], [1, NCLS]]),
                    AP(fac2[:], 0, [[OCH, BL], [1, OCH], [0, NCLS]]))
                nc.sync.dma_start(y[:], yout[:])
            else:
                nc.vector.tensor_mul(
                    o3bf[:],
                    AP(s_sb[:], 0, [[NF, BL], [1, NF], [0, 1]]),
                    AP(fac2[:], 0, facap))
                # G^T = sum_b out (x) u ; E = G^T . W3  (per 1152-col chunk)
                with tc.tile_pool(name=f"psG{it}", bufs=2, space="PSUM") as psG:
                    for ci in range(8):
                        sl = slice(ci * NI, (ci + 1) * NI)
                        ga = psG.tile([128, NI], F32, tag="g", name=f"ga{it}_{ci}")
                        for (n0, n1) in ((0, 512), (512, 1024), (1024, 1152)):
                            nc.tensor.matmul(
                                ga[:, n0:n1], lhsT=o3bf[:, 0:128],
                                rhs=u_bT[:, ci * NI + n0:ci * NI + n1],
                                start=True, stop=True)
                        if ci in (0, 3, 6):
                            nc.vector.tensor_mul(e2a[:, sl], ga[:], w3a_sb[:, sl])
                        else:
                            gbf = gbf0 if ci % 2 == 1 else gbf1
                            nc.scalar.copy(gbf[:], ga[:])
                            nc.vector.tensor_mul(e2a[:, sl], gbf[:], w3a_sb[:, sl])
                        gb = psG.tile([128, NI], F32, tag="g", name=f"gb{it}_{ci}")
                        for (n0, n1) in ((0, 512), (512, 1024), (1024, 1152)):
                            nc.tensor.matmul(
                                gb[0:32, n0:n1], lhsT=o3bf[:, 128:160],
                                rhs=u_bT[:, ci * NI + n0:ci * NI + n1],
                                start=True, stop=True)
                        if ci in (1, 4, 7):
                            nc.vector.tensor_mul(e2b[:, sl], gb[0:32, :],
                                                 w3b_sb[:, sl])
                        else:
                            gbfb = gbfb0 if ci % 2 == 1 else gbfb1
                            nc.scalar.copy(gbfb[:], gb[0:32, :])
                            nc.vector.tensor_mul(e2b[:, sl], gbfb[:],
                                                 w3b_sb[:, sl])
                # u_v partial: select classes, fold k, transpose to b-layout
                with tc.tile_pool(name=f"psU{it}", bufs=1, space="PSUM") as psU:
                    puv = psU.tile([NCLS, NI], F32)
                    for kc, (ss, e2) in enumerate(((ssa_sb, e2a), (ssb_sb, e2b))):
                        for k in range(8):
                            for (n0, n1) in ((0, 512), (512, 1024), (1024, 1152)):
                                nc.tensor.matmul(
                                    puv[:, n0:n1], lhsT=ss[:],
                                    rhs=e2[:, k * NI + n0:k * NI + n1],
                                    start=(kc == 0 and k == 0),
                                    stop=(kc == 1 and k == 7))
                    # reorder (chw, posj, q) -> (posj, chw, q) during psum copy
                    nc.scalar.copy(
                        uvs[:],
                        AP(puv[:], 0, [[NI, NCLS], [4, 9], [36, 32], [1, 4]]))
                    uvT = psU.tile([128, 9, NCLS], F32)
                    for pj in range(9):
                        nc.tensor.transpose(
                            uvT[:, pj, :],
                            uvs[:, pj * 128:(pj + 1) * 128],
                            ident[0:NCLS, 0:NCLS])
                    nc.vector.tensor_copy(uvsb[:], uvT[:].rearrange("p j c -> p (j c)"))
                nc.sync.dma_start(agin[:], uvsb[:])
                nc.gpsimd.collective_compute(
                    "AllGather", mybir.AluOpType.bypass,
                    replica_groups=[list(range(N_CORES))],
                    ins=[agin.opt()], outs=[agout.opt()])
                nc.sync.dma_start(
                    gth[:], AP(agout[:], 0, [[90, 128], [11520, N_CORES], [1, 90]]))
                nc.vector.reduce_sum(
                    out=uvred[:],
                    in_=AP(gth[:], 0, [[N_CORES * 90, 128], [1, 90], [90, N_CORES]]),
                    axis=mybir.AxisListType.X)
                nc.vector.tensor_add(b2t[:], b2t[:], uvred[:])

    nc.compile()
    return nc


_CACHE = {}


def _get_program():
    if "nc" not in _CACHE:
        _CACHE["nc"] = build_program()
    return _CACHE["nc"]


def _host_inputs(x, conv_w, conv_b, prim_w, prim_b, digit_w):
    x = np.asarray(x, dtype=np.float32)
    conv_w = np.asarray(conv_w, dtype=np.float32)
    conv_b = np.asarray(conv_b, dtype=np.float32)
    prim_w = np.asarray(prim_w, dtype=np.float32)
    prim_b = np.asarray(prim_b, dtype=np.float32)
    digit_w = np.asarray(digit_w, dtype=np.float32)
    bf16 = ml_dtypes.bfloat16

    # im2col of x: (B, 1, 28, 28) -> (B, 81, 400) windows
    xi = x.reshape(B, 28, 28)
    s0, s1, s2 = xi.strides
    win = np.lib.stride_tricks.as_strided(
        xi, shape=(B, 9, 9, 20, 20), strides=(s0, s1, s2, s1, s2))
    icold_full = np.ascontiguousarray(
        win.reshape(B, 81, 400).transpose(1, 0, 2))      # (81, B, 400)

    w1 = np.ascontiguousarray(conv_w.reshape(256, 81).T).astype(bf16)
    b1 = np.ascontiguousarray(conv_b.reshape(2, 128).T)
    # natural oc order: oc = t*128+p, k = oc//32, chw = oc%32
    w2 = np.ascontiguousarray(prim_w.reshape(256, 256, 81).transpose(2, 1, 0))
    b2 = np.ascontiguousarray(prim_b.reshape(2, 128).T)

    p_ = np.arange(128)
    # wr[p, t, pos, o, c] = digit_w[i, c, o, k]; i=(p%32)*36+pos, k=(t*128+p)//32
    dw_ikoc = digit_w.transpose(0, 3, 2, 1)              # (i, k, o, c)
    i_pp = (p_[:, None] % 32) * 36 + np.arange(36)[None, :]      # (128, 36)
    k_pt = (np.arange(2)[None, :] * 128 + p_[:, None]) // 32     # (128, 2)
    wr = dw_ikoc[i_pp[:, None, :], k_pt[:, :, None], :, :]       # (128,2,36,16,10)
    wr = np.ascontiguousarray(wr.reshape(128, 2 * 36 * NF)).astype(bf16)

    # w3[f=(o,c), r=(oc,pos)] = digit_w[i(r), c(f), o(f), k(r)]
    r_ = np.arange(NR)
    oc_r = r_ // 36
    i_r = (oc_r % 32) * 36 + (r_ % 36)
    k_r = oc_r // 32
    dw_oci = digit_w.transpose(2, 1, 0, 3)               # (o, c, i, k)
    w3 = dw_oci[:, :, i_r, k_r].reshape(NF, NR)          # (160, 9216)
    w3a = np.ascontiguousarray(w3[0:128]).astype(bf16)
    w3b = np.ascontiguousarray(w3[128:160]).astype(bf16)

    # class selector (sums over o within class, folds 1/B)
    f_ = np.arange(NF)
    ssel = np.zeros((NF, NCLS), np.float32)
    ssel[f_, f_ % NCLS] = 1.0 / B
    ssa = ssel[0:128].astype(bf16)
    ssb = ssel[128:160].astype(bf16)

    capsum = np.zeros((128, 2, 8), np.float32)
    expnd = np.zeros((8, 2, 128), np.float32)
    for t in range(2):
        cap = (t * 128 + p_) // 32
        capsum[p_, t, cap] = 1.0
        expnd[cap, t, p_] = 1.0

    # selq[pq, q, p] = 1 iff pq == (p%32)*4 + q
    selq = np.zeros((128, 4, 128), np.float32)
    for q in range(4):
        selq[(p_ % 32) * 4 + q, q, p_] = 1.0
    selq = selq.astype(bf16)

    ones128 = np.ones((128, 1), np.float32)
    ones1 = np.ones((1, 128), np.float32)

    shared = {
        "w1": w1, "b1": b1, "w2": w2, "b2": b2,
        "wr": wr, "w3a": w3a, "w3b": w3b, "ssa": ssa, "ssb": ssb,
        "capsum": capsum, "expnd": expnd, "selq": selq,
        "ones128": ones128, "ones1": ones1,
    }
    icold_bf = icold_full.astype(bf16)
    in_maps = []
    for m in range(N_CORES):
        d = dict(shared)
        d["icold"] = np.ascontiguousarray(icold_bf[:, m * BL:(m + 1) * BL, :])
        in_maps.append(d)
    return in_maps


def kernel(x, conv_w, conv_b, prim_w, prim_b, digit_w, trace=False):
    nc = _get_program()
    in_maps = _host_inputs(x, conv_w, conv_b, prim_w, prim_b, digit_w)
    res = run_bass_kernel_spmd(nc, in_maps, list(range(N_CORES)), trace=trace)
    out = np.concatenate(
        [np.asarray(res.results[m]["y"]).reshape(BL, NCLS, OCH)
         for m in range(N_CORES)], axis=0)[..., None].astype(np.float32)
    if trace:
        return out, res
    return out


# revision 34
# speedup vs baseline: 1.2789x; 1.0061x over previous
"""CapsNet Trainium2 kernel: conv stack + primary caps + dynamic routing.

Distribution: pure batch-parallel (32 images/core), replicated routing
weights. b_ij is kept replicated on every core, so s_j/out are computed
locally per core for its own images; the only communication is one
AllGather of the per-core partial agreement updates u_v per routing
iteration (2 total — the final iteration needs none, each core writes its
own y chunk and the host concatenates).

Layout conventions on device (per core):
  h      [ic_p 128, ic_t 2, b 32, y 20, x 20]     conv1 out / conv2 in (f32r)
  u      [oc_p 128, oc_t 2, b 32, pos 36]          conv2 out, natural oc order
                                                   (oc = t*128+p, k=oc//32, chw=oc%32)
  r      = oc*36 + pos = k*1152 + i,  i = chw*36 + pos   routing row index
  f      = o*10 + c   class dims ordered o-major (enables packed-c DVE APs)
  u_bT   [b 32, r 9216] bf16                       via DRAM roundtrip
  b_ij   [pq 128, posj 9, c 10]  pq = chw*4 + pos%4, posj = pos//4
"""

import numpy as np
import ml_dtypes
from contextlib import ExitStack

import concourse.bass as bass
import concourse.tile as tile
from concourse import bacc, mybir
from concourse.bass_utils import run_bass_kernel_spmd
from concourse.masks import make_identity

F32 = mybir.dt.float32
F32R = mybir.dt.float32r
BF16 = mybir.dt.bfloat16
AF = mybir.ActivationFunctionType

N_CORES = 8
B = 256
BL = B // N_CORES          # 32 local batch
NCLS = 10
OCH = 16
NF = 160                   # (o, c) o-major
NR = 9216                  # routing rows r = (oc, pos)
NI = 1152                  # capsule units per capsule map
NITER = 3


def AP(t_ap, off, dims):
    return bass.AP(tensor=t_ap.tensor, offset=t_ap.offset + off,
                   ap=[list(d) for d in dims])


def build_program():
    nc = bacc.Bacc("TRN2", target_bir_lowering=False, debug=False,
                   num_devices=N_CORES)

    icold = nc.dram_tensor("icold", [81, BL, 400], BF16, kind="ExternalInput")
    w1 = nc.dram_tensor("w1", [81, 256], BF16, kind="ExternalInput")
    b1 = nc.dram_tensor("b1", [128, 2], F32, kind="ExternalInput")
    w2 = nc.dram_tensor("w2", [81, 256, 256], F32, kind="ExternalInput")
    b2 = nc.dram_tensor("b2", [128, 2], F32, kind="ExternalInput")
    wr = nc.dram_tensor("wr", [128, 2 * 36 * NF], BF16, kind="ExternalInput")
    w3a = nc.dram_tensor("w3a", [128, NR], BF16, kind="ExternalInput")
    w3b = nc.dram_tensor("w3b", [32, NR], BF16, kind="ExternalInput")
    ssa = nc.dram_tensor("ssa", [128, NCLS], BF16, kind="ExternalInput")
    ssb = nc.dram_tensor("ssb", [32, NCLS], BF16, kind="ExternalInput")
    capsum = nc.dram_tensor("capsum", [128, 2, 8], F32, kind="ExternalInput")
    expnd = nc.dram_tensor("expnd", [8, 2, 128], F32, kind="ExternalInput")
    selq = nc.dram_tensor("selq", [128, 4, 128], BF16, kind="ExternalInput")
    ones128 = nc.dram_tensor("ones128", [128, 1], F32, kind="ExternalInput")
    ones1 = nc.dram_tensor("ones1", [1, 128], F32, kind="ExternalInput")
    y = nc.dram_tensor("y", [BL, NF], F32, kind="ExternalOutput")

    with tile.TileContext(nc) as tc, ExitStack() as ctx:
        consts = ctx.enter_context(tc.tile_pool(name="consts", bufs=1))
        dram = ctx.enter_context(tc.tile_pool(name="dram", bufs=1, space="DRAM"))

        b1_sb = consts.tile([128, 2], F32)
        nc.sync.dma_start(b1_sb[:], b1[:])
        b2_sb = consts.tile([128, 2], F32)
        nc.sync.dma_start(b2_sb[:], b2[:])
        capsum_sb = consts.tile([128, 2, 8], F32)
        nc.sync.dma_start(capsum_sb[:], capsum[:])
        expnd_sb = consts.tile([8, 2, 128], F32)
        nc.sync.dma_start(expnd_sb[:], expnd[:])
        selq_sb = consts.tile([128, 4, 128], BF16)
        nc.sync.dma_start(selq_sb[:], selq[:])
        ssa_sb = consts.tile([128, NCLS], BF16)
        nc.sync.dma_start(ssa_sb[:], ssa[:])
        ssb_sb = consts.tile([32, NCLS], BF16)
        nc.sync.dma_start(ssb_sb[:], ssb[:])
        ones128_sb = consts.tile([128, 1], F32)
        nc.sync.dma_start(ones128_sb[:], ones128[:])
        ones1_sb = consts.tile([1, 128], F32)
        nc.sync.dma_start(ones1_sb[:], ones1[:])
        ident = consts.tile([128, 128], F32)
        make_identity(nc, ident[:])

        ustage = dram.tile([BL, NR], BF16)
        agin = dram.tile([128, 90], BF16)
        agout = dram.tile([N_CORES, 128, 90], BF16)

        upool = ctx.enter_context(tc.tile_pool(name="upool", bufs=1))
        u = upool.tile([128, 2, BL, 36], F32)
        usq = upool.tile([128, 2, BL * 36], F32)

        # routing weights: allocated before hpool, streamed during conv
        rw = ctx.enter_context(tc.tile_pool(name="rw", bufs=1))
        wr_sb = rw.tile([128, 2 * 36 * NF], BF16)
        w3a_sb = rw.tile([128, NR], BF16)
        w3b_sb = rw.tile([32, NR], BF16)
        wrv = wr_sb[:].rearrange("p (t s f) -> p t s f", t=2, s=36)

        with tc.tile_pool(name="hpool", bufs=1) as hpool:
            h = hpool.tile([128, 2, 32, 20, 20], F32R)

            # ============ conv1: 1->256 k9 s1 + ReLU (bf16 inputs) ============
            with tc.tile_pool(name="icolp", bufs=1) as icolp, \
                 tc.tile_pool(name="ps1", bufs=1, space="PSUM") as ps1:
                w1_sb = icolp.tile([81, 256], BF16)
                nc.sync.dma_start(w1_sb[:], w1[:])
                icol_bf = icolp.tile([81, BL, 400], BF16)
                for bc in range(4):
                    nc.sync.dma_start(icol_bf[:, bc * 8:(bc + 1) * 8, :],
                                      icold[:][:, bc * 8:(bc + 1) * 8, :])
                # 4-image quads per psum tile -> 4x fewer relu ops
                for t in range(2):
                    for bq in range(8):
                        p = ps1.tile([128, 4, 512], F32, tag="c1", bufs=2,
                                     name=f"c1_{t}_{bq}")
                        for j in range(4):
                            nc.tensor.matmul(
                                p[:, j, 0:400],
                                lhsT=w1_sb[:, t * 128:(t + 1) * 128],
                                rhs=icol_bf[:, bq * 4 + j, :],
                                start=True, stop=True)
                        pin = AP(p[:], 0,
                                 [[2048, 128], [512, 4], [20, 20], [1, 20]])
                        hout = h[:, t, bq * 4:bq * 4 + 4, :, :]
                        if bq % 2 == 0:
                            nc.scalar.activation(
                                hout, pin, AF.Relu,
                                bias=b1_sb[:, t:t + 1], scale=1.0)
                        else:
                            nc.vector.tensor_scalar(
                                out=hout, in0=pin,
                                scalar1=b1_sb[:, t:t + 1], scalar2=0.0,
                                op0=mybir.AluOpType.add,
                                op1=mybir.AluOpType.max)

            # routing weight DMAs (overlap conv2)
            nc.sync.dma_start(wr_sb[:, 0:5760], wr[:][:, 0:5760])
            nc.sync.dma_start(wr_sb[:, 5760:11520], wr[:][:, 5760:11520])
            nc.sync.dma_start(w3a_sb[:], w3a[:])
            nc.sync.dma_start(w3b_sb[:], w3b[:])

            # ============ conv2: 256->256 k9 s2 + bias ============
            hv = h[:]
            with tc.tile_pool(name="w2p", bufs=3) as w2p, \
                 tc.tile_pool(name="ps2", bufs=8, space="PSUM") as ps2:
                psum2 = [ps2.tile([128, 8, 36], F32, tag="c2", name=f"c2_{i}")
                         for i in range(8)]
                for kk in range(81):
                    ky, kx = kk // 9, kk % 9
                    w2t = w2p.tile([128, 2, 256], F32)
                    nc.sync.dma_start(
                        w2t[:],
                        AP(w2[:], kk * 65536, [[256, 128], [32768, 2], [1, 256]]))
                    w2r = w2p.tile([128, 2, 256], F32R)
                    nc.vector.tensor_copy(w2r[:], w2t[:])
                    for ic_t in range(2):
                        for oc_t in range(2):
                            lhs = w2r[:, ic_t, oc_t * 128:(oc_t + 1) * 128]
                            for bc in range(4):
                                rhs = hv[:, ic_t, bc * 8:(bc + 1) * 8,
                                         ky:ky + 12:2, kx:kx + 12:2]
                                nc.tensor.matmul(
                                    psum2[oc_t * 4 + bc][:], lhsT=lhs,
                                    rhs=rhs,
                                    start=(kk == 0 and ic_t == 0),
                                    stop=(kk == 80 and ic_t == 1))
                # u = psum+bias on DVE; u^2 = (psum+bias)^2 on Act, in parallel
                for oc_t in range(2):
                    for bc in range(4):
                        ps = psum2[oc_t * 4 + bc]
                        nc.vector.tensor_scalar_add(
                            u[:, oc_t, bc * 8:(bc + 1) * 8, :], ps[:],
                            b2_sb[:, oc_t:oc_t + 1])
                        nc.scalar.activation(
                            usq[:, oc_t, bc * 288:(bc + 1) * 288]
                            .rearrange("p (b q) -> p b q", q=36),
                            ps[:], AF.Square,
                            bias=b2_sb[:, oc_t:oc_t + 1], scale=1.0)
        rts = ctx.enter_context(tc.tile_pool(name="rts", bufs=1))
        u_bf = rts.tile([128, 2, BL, 36], BF16)

        # ============ squash over i per (b, cap) ============
        with tc.tile_pool(name="sqp", bufs=1) as sqp, \
             tc.tile_pool(name="psq", bufs=1, space="PSUM") as psq:
            pnorm = psq.tile([8, BL * 36], F32)
            for t in range(2):
                for (n0, n1) in ((0, 512), (512, 1024), (1024, 1152)):
                    nc.tensor.matmul(pnorm[:, n0:n1], lhsT=capsum_sb[:, t, :],
                                     rhs=usq[:, t, n0:n1],
                                     start=(t == 0), stop=(t == 1))
            normsq = sqp.tile([8, BL], F32)
            nc.vector.reduce_sum(
                out=normsq[:],
                in_=pnorm[:].rearrange("c (b q) -> c b q", q=36),
                axis=mybir.AxisListType.X)
            # fac = sqrt(n)/(1+n) = exp(0.5*ln(n) - ln(1+n)); avoids the
            # sqrt act-table (stays in the ln/exp table set all routing long)
            lnn = sqp.tile([8, BL], F32)
            nc.scalar.activation(lnn[:], normsq[:], AF.Ln)
            ln1 = sqp.tile([8, BL], F32)
            nc.scalar.activation(ln1[:], normsq[:], AF.Ln, bias=1.0)
            wv = sqp.tile([8, BL], F32)
            nc.vector.tensor_scalar_mul(wv[:], lnn[:], 0.5)
            nc.vector.tensor_sub(wv[:], wv[:], ln1[:])
            fac = sqp.tile([8, BL], F32)
            nc.scalar.activation(fac[:], wv[:], AF.Exp)
            sfac = sqp.tile([128, 2, BL], F32)
            for t in range(2):
                pfac = psq.tile([128, BL], F32, tag="pf", name=f"pf{t}")
                nc.tensor.matmul(pfac[:], lhsT=expnd_sb[:, t, :],
                                 rhs=fac[:], start=True, stop=True)
                nc.scalar.copy(sfac[:, t, :], pfac[:])
                # u_bf = squashed u in bf16 (single fused scale+cast)
                nc.vector.tensor_mul(
                    u_bf[:, t, :, :], u[:, t, :, :],
                    AP(sfac[:], t * BL, [[2 * BL, 128], [1, BL], [0, 36]]))

        # ============ u_bT via DRAM roundtrip ============
        for t in range(2):
            nc.sync.dma_start(
                AP(ustage[:], t * 4608, [[36, 128], [NR, BL], [1, 36]]),
                u_bf[:, t, :, :])
        u_bT = rts.tile([BL, NR], BF16)
        nc.sync.dma_start(u_bT[:], ustage[:])

        # ============ routing state ============
        b2t = rts.tile([128, 90], F32)
        nc.vector.memset(b2t[:], 0.0)
        e2a = rts.tile([128, NR], BF16)
        e2b = rts.tile([32, NR], BF16)
        ew = rts.tile([128, 2, 36, NF], BF16)
        expb = rts.tile([128, 90], F32)
        ez = rts.tile([128, 90], F32)
        expzb = rts.tile([128, 90], BF16)
        e_sb = rts.tile([128, 360], BF16)
        zrow = rts.tile([1, NCLS], F32)
        rz = rts.tile([1, NCLS], F32)
        rz32s = rts.tile([BL, NCLS], F32)
        s_sb = rts.tile([BL, NF], F32)
        sq2 = rts.tile([BL, NF], F32)
        nrm = rts.tile([BL, OCH], F32)
        scl2 = rts.tile([BL, OCH], F32)
        onep2 = rts.tile([BL, OCH], F32)
        den2 = rts.tile([BL, OCH], F32)
        rden2 = rts.tile([BL, OCH], F32)
        fac2 = rts.tile([BL, OCH], F32)
        o3bf = rts.tile([BL, NF], BF16)
        yout = rts.tile([BL, NF], F32)
        uvs = rts.tile([NCLS, NI], F32)
        uvsb = rts.tile([128, 90], BF16)
        gth = rts.tile([128, N_CORES, 90], BF16)
        uvred = rts.tile([128, 90], F32)
        gbf0 = rts.tile([128, NI], BF16)
        gbf1 = rts.tile([128, NI], BF16)
        gbfb0 = rts.tile([32, NI], BF16)
        gbfb1 = rts.tile([32, NI], BF16)

        facap = [[OCH, BL], [1, OCH], [0, NCLS]]
        s3ap = [[NF, BL], [NCLS, OCH], [1, NCLS]]

        for it in range(NITER):
            if it > 0:
                # c_ij = softmax(b) over all i; fold 1/Z into the exp table
                with tc.tile_pool(name=f"psE{it}", bufs=1, space="PSUM") as psE:
                    nc.scalar.activation(expb[:], b2t[:], AF.Exp)
                    psz = psE.tile([1, 90], F32)
                    nc.tensor.matmul(psz[:], lhsT=ones128_sb[:], rhs=expb[:],
                                     start=True, stop=True)
                    nc.vector.reduce_sum(
                        out=zrow[:],
                        in_=AP(psz[:], 0, [[90, 1], [1, NCLS], [NCLS, 9]]),
                        axis=mybir.AxisListType.X)
                    nc.vector.reciprocal(rz[:], zrow[:])
                    # keep exp(b) unnormalized; 1/Z folds into the s_j readout
                    nc.vector.tensor_copy(expzb[:], expb[:])
                    # expand (pq, posj, c) -> (p, pos, c): e_ps[q][p, posj, c]
                    e_ps = psE.tile([128, 4, 90], F32)
                    for q in range(4):
                        nc.tensor.matmul(e_ps[:, q, :], lhsT=selq_sb[:, q, :],
                                         rhs=expzb[:], start=True, stop=True)
                    nc.scalar.copy(
                        e_sb[:],
                        AP(e_ps[:], 0, [[360, 128], [NCLS, 9], [90, 4], [1, NCLS]]))
                # ew[p, t, pos, o, c] = W * c_ij  (all-bf16 SBUF fast path);
                # chunked so the s_j matmuls below start on chunk 0 early
                for t in range(2):
                    for hh in range(2):
                        ps0 = hh * 18
                        nc.vector.tensor_mul(
                            ew[:, t, ps0:ps0 + 18, :],
                            wrv[:, t, ps0:ps0 + 18, :],
                            AP(e_sb[:], ps0 * NCLS,
                               [[360, 128], [NCLS, 18], [0, OCH], [1, NCLS]]))
            rhs_w = wrv if it == 0 else ew[:]

            # s_j for local batch: accumulate over all (oc, pos) rows
            with tc.tile_pool(name=f"psS{it}", bufs=1, space="PSUM") as psS:
                psj = psS.tile([BL, NF], F32)
                if it > 0:
                    rz32 = psS.tile([BL, NCLS], F32)
                    nc.tensor.matmul(rz32[:], lhsT=ones1_sb[0:1, 0:BL],
                                     rhs=rz[:], start=True, stop=True)
                    nc.scalar.copy(rz32s[:], rz32[:])
                for t in range(2):
                    for pos in range(36):
                        nc.tensor.matmul(
                            psj[:], lhsT=u_bf[:, t, :, pos],
                            rhs=rhs_w[:, t, pos, :],
                            start=(t == 0 and pos == 0),
                            stop=(t == 1 and pos == 35))
                if it == 0:
                    nc.vector.tensor_scalar_mul(s_sb[:], psj[:], 1.0 / NI)
                else:
                    nc.vector.tensor_mul(
                        s_sb[:], psj[:],
                        AP(rz32s[:], 0, [[NCLS, BL], [0, OCH], [1, NCLS]]))
            # squash over classes per (b, o): fac = sqrt(n)/(1+n)
            nc.vector.tensor_mul(sq2[:], s_sb[:], s_sb[:])
            nc.vector.reduce_sum(out=nrm[:], in_=AP(sq2[:], 0, s3ap),
                                 axis=mybir.AxisListType.X)
            nc.scalar.activation(scl2[:], nrm[:], AF.Ln)
            nc.scalar.activation(onep2[:], nrm[:], AF.Ln, bias=1.0)
            nc.vector.tensor_scalar_mul(den2[:], scl2[:], 0.5)
            nc.vector.tensor_sub(den2[:], den2[:], onep2[:])
            nc.scalar.activation(fac2[:], den2[:], AF.Exp)

            if it == NITER - 1:
                # out in (c, o) order -> y chunk; host concatenates cores
                nc.vector.tensor_mul(
                    AP(yout[:], 0, [[NF, BL], [1, OCH], [OCH, NCLS]]),
                    AP(s_sb[:], 0, [[NF, BL], [NCLS, OCH], [1, NCLS]]),
                    AP(fac2[:], 0, [[OCH, BL], [1, OCH], [0, NCLS]]))
                nc.sync.dma_start(y[:], yout[:])
            else:
                nc.vector.tensor_mul(
                    o3bf[:],
                    AP(s_sb[:], 0, [[NF, BL], [1, NF], [0, 1]]),
                    AP(fac2[:], 0, facap))
                # G^T = sum_b out (x) u ; E = G^T . W3  (per 1152-col chunk)
                with tc.tile_pool(name=f"psG{it}", bufs=2, space="PSUM") as psG:
                    for ci in range(8):
                        sl = slice(ci * NI, (ci + 1) * NI)
                        ga = psG.tile([128, NI], F32, tag="g", name=f"ga{it}_{ci}")
                        for (n0, n1) in ((0, 512), (512, 1024), (1024, 1152)):
                            nc.tensor.matmul(
                                ga[:, n0:n1], lhsT=o3bf[:, 0:128],
                                rhs=u_bT[:, ci * NI + n0:ci * NI + n1],
                                start=True, stop=True)
                        if ci in (0, 3, 6):
                            nc.vector.tensor_mul(e2a[:, sl], ga[:], w3a_sb[:, sl])
                        else:
                            gbf = gbf0 if ci % 2 == 1 else gbf1
                            nc.scalar.copy(gbf[:], ga[:])
                            nc.vector.tensor_mul(e2a[:, sl], gbf[:], w3a_sb[:, sl])
                        gb = psG.tile([128, NI], F32, tag="g", name=f"gb{it}_{ci}")
                        for (n0, n1) in ((0, 512), (512, 1024), (1024, 1152)):
                            nc.tensor.matmul(
                                gb[0:32, n0:n1], lhsT=o3bf[:, 128:160],
                                rhs=u_bT[:, ci * NI + n0:ci * NI + n1],
                                start=True, stop=True)
                        if ci in (1, 4, 7):
                            nc.vector.tensor_mul(e2b[:, sl], gb[0:32, :],
                                                 w3b_sb[:, sl])
                        else:
                            gbfb = gbfb0 if ci % 2 == 1 else gbfb1
                            nc.scalar.copy(gbfb[:], gb[0:32, :])
                            nc.vector.tensor_mul(e2b[:, sl], gbfb[:],
                                                 w3b_sb[:, sl])
                # u_v partial: select classes, fold k, transpose to b-layout
                with tc.tile_pool(name=f"psU{it}", bufs=1, space="PSUM") as psU:
                    puv = psU.tile([NCLS, NI], F32)
                    for kc, (ss, e2) in enumerate(((ssa_sb, e2a), (ssb_sb, e2b))):
                        for k in range(8):
                            for (n0, n1) in ((0, 512), (512, 1024), (1024, 1152)):
                                nc.tensor.matmul(
                                    puv[:, n0:n1], lhsT=ss[:],
                                    rhs=e2[:, k * NI + n0:k * NI + n1],
                                    start=(kc == 0 and k == 0),
                                    stop=(kc == 1 and k == 7))
                    # reorder (chw, posj, q) -> (posj, chw, q) during psum copy
                    nc.scalar.copy(
                        uvs[:],
                        AP(puv[:], 0, [[NI, NCLS], [4, 9], [36, 32], [1, 4]]))
                    uvT = psU.tile([128, 9, NCLS], F32)
                    for pj in range(9):
                        nc.tensor.transpose(
                            uvT[:, pj, :],
                            uvs[:, pj * 128:(pj + 1) * 128],
                            ident[0:NCLS, 0:NCLS])
                    nc.vector.tensor_copy(uvsb[:], uvT[:].rearrange("p j c -> p (j c)"))
                nc.sync.dma_start(agin[:], uvsb[:])
                nc.gpsimd.collective_compute(
                    "AllGather", mybir.AluOpType.bypass,
                    replica_groups=[list(range(N_CORES))],
                    ins=[agin.opt()], outs=[agout.opt()])
                nc.sync.dma_start(
                    gth[:], AP(agout[:], 0, [[90, 128], [11520, N_CORES], [1, 90]]))
                nc.vector.reduce_sum(
                    out=uvred[:],
                    in_=AP(gth[:], 0, [[N_CORES * 90, 128], [1, 90], [90, N_CORES]]),
                    axis=mybir.AxisListType.X)
                nc.vector.tensor_add(b2t[:], b2t[:], uvred[:])

    nc.compile()
    return nc


_CACHE = {}


def _get_program():
    if "nc" not in _CACHE:
        _CACHE["nc"] = build_program()
    return _CACHE["nc"]


def _host_inputs(x, conv_w, conv_b, prim_w, prim_b, digit_w):
    x = np.asarray(x, dtype=np.float32)
    conv_w = np.asarray(conv_w, dtype=np.float32)
    conv_b = np.asarray(conv_b, dtype=np.float32)
    prim_w = np.asarray(prim_w, dtype=np.float32)
    prim_b = np.asarray(prim_b, dtype=np.float32)
    digit_w = np.asarray(digit_w, dtype=np.float32)
    bf16 = ml_dtypes.bfloat16

    # im2col of x: (B, 1, 28, 28) -> (B, 81, 400) windows
    xi = x.reshape(B, 28, 28)
    s0, s1, s2 = xi.strides
    win = np.lib.stride_tricks.as_strided(
        xi, shape=(B, 9, 9, 20, 20), strides=(s0, s1, s2, s1, s2))
    icold_full = np.ascontiguousarray(
        win.reshape(B, 81, 400).transpose(1, 0, 2))      # (81, B, 400)

    w1 = np.ascontiguousarray(conv_w.reshape(256, 81).T).astype(bf16)
    b1 = np.ascontiguousarray(conv_b.reshape(2, 128).T)
    # natural oc order: oc = t*128+p, k = oc//32, chw = oc%32
    w2 = np.ascontiguousarray(prim_w.reshape(256, 256, 81).transpose(2, 1, 0))
    b2 = np.ascontiguousarray(prim_b.reshape(2, 128).T)

    p_ = np.arange(128)
    # wr[p, t, pos, o, c] = digit_w[i, c, o, k]; i=(p%32)*36+pos, k=(t*128+p)//32
    dw_ikoc = digit_w.transpose(0, 3, 2, 1)              # (i, k, o, c)
    i_pp = (p_[:, None] % 32) * 36 + np.arange(36)[None, :]      # (128, 36)
    k_pt = (np.arange(2)[None, :] * 128 + p_[:, None]) // 32     # (128, 2)
    wr = dw_ikoc[i_pp[:, None, :], k_pt[:, :, None], :, :]       # (128,2,36,16,10)
    wr = np.ascontiguousarray(wr.reshape(128, 2 * 36 * NF)).astype(bf16)

    # w3[f=(o,c), r=(oc,pos)] = digit_w[i(r), c(f), o(f), k(r)]
    r_ = np.arange(NR)
    oc_r = r_ // 36
    i_r = (oc_r % 32) * 36 + (r_ % 36)
    k_r = oc_r // 32
    dw_oci = digit_w.transpose(2, 1, 0, 3)               # (o, c, i, k)
    w3 = dw_oci[:, :, i_r, k_r].reshape(NF, NR)          # (160, 9216)
    w3a = np.ascontiguousarray(w3[0:128]).astype(bf16)
    w3b = np.ascontiguousarray(w3[128:160]).astype(bf16)

    # class selector (sums over o within class, folds 1/B)
    f_ = np.arange(NF)
    ssel = np.zeros((NF, NCLS), np.float32)
    ssel[f_, f_ % NCLS] = 1.0 / B
    ssa = ssel[0:128].astype(bf16)
    ssb = ssel[128:160].astype(bf16)

    capsum = np.zeros((128, 2, 8), np.float32)
    expnd = np.zeros((8, 2, 128), np.float32)
    for t in range(2):
        cap = (t * 128 + p_) // 32
        capsum[p_, t, cap] = 1.0
        expnd[cap, t, p_] = 1.0

    # selq[pq, q, p] = 1 iff pq == (p%32)*4 + q
    selq = np.zeros((128, 4, 128), np.float32)
    for q in range(4):
        selq[(p_ % 32) * 4 + q, q, p_] = 1.0
    selq = selq.astype(bf16)

    ones128 = np.ones((128, 1), np.float32)
    ones1 = np.ones((1, 128), np.float32)

    shared = {
        "w1": w1, "b1": b1, "w2": w2, "b2": b2,
        "wr": wr, "w3a": w3a, "w3b": w3b, "ssa": ssa, "ssb": ssb,
        "capsum": capsum, "expnd": expnd, "selq": selq,
        "ones128": ones128, "ones1": ones1,
    }
    icold_bf = icold_full.astype(bf16)
    in_maps = []
    for m in range(N_CORES):
        d = dict(shared)
        d["icold"] = np.ascontiguousarray(icold_bf[:, m * BL:(m + 1) * BL, :])
        in_maps.append(d)
    return in_maps


def kernel(x, conv_w, conv_b, prim_w, prim_b, digit_w, trace=False):
    nc = _get_program()
    in_maps = _host_inputs(x, conv_w, conv_b, prim_w, prim_b, digit_w)
    res = run_bass_kernel_spmd(nc, in_maps, list(range(N_CORES)), trace=trace)
    out = np.concatenate(
        [np.asarray(res.results[m]["y"]).reshape(BL, NCLS, OCH)
         for m in range(N_CORES)], axis=0)[..., None].astype(np.float32)
    if trace:
        return out, res
    return out
